# revision 36
# baseline (speedup 1.0000x reference)
"""Chamfer distance kernel for Trainium2 (8 NeuronCores, Bass/Tile).

Problem: p1, p2 are [B=8, N=4096, D=3] fp32 point clouds. Output is the
scalar  mean_j(min_i P[b,i,j]) + mean_i(min_j P[b,i,j])  where
P[b,i,j] = ||p1[b,i] - p2[b,j]||^2.

Strategy
--------
Data-parallel over B: core b handles batch b.

Nearest-neighbor structure: on the host each batch's points are sorted by
coordinate 0. Nearest neighbors are then close in *rank*, so instead of the
full [N, N] distance matrix each 128-point block only scans a W-wide window
of rank-adjacent candidates (a banded distance matrix). Both directions
(min over rows / min over cols) are computed as separate banded passes with
the roles of the two point sets swapped, so on-device both reductions are
free-axis `tensor_reduce(min)` ops.

The distance block is a single K=5 matmul via the augmentation
  lhsT rows = [x0, x1, x2, ||x||^2/2, 1]
  rhs  rows = [-y0, -y1, -y2, 1, ||y||^2/2]
giving P/2 per element; row mins are doubled on the host.

Exactness: banding alone can miss isolated points. For each row the host
runs an O(1) posterior bound check — every candidate outside the window has
dist^2 >= (coord0 gap to the window edge)^2, so a row whose banded min is
below that gap is *provably* exact. The few unproven rows (~0.6% on
randn data) are recomputed exactly on the host with a full scan.
"""

import sys

import numpy as np

if "/opt/trn_rl_repo" not in sys.path:
    sys.path.insert(0, "/opt/trn_rl_repo")

B = 8
N = 4096
D = 3
W = 352          # band width (candidates per 128-row block)
WPAD = 512       # PSUM bank stride per block (fp32 elems; 2KB bank)
NBLK = N // 128  # 32 row blocks per side
GROUP = 4        # blocks reduced per tensor_reduce (4 PSUM banks)
N_CORES = 8
KAUG = 24        # bf16-split augmented contraction dim (see _aug_pair)
WARMG = 4        # leading x-side groups served from the warm-start chunk
WARM_LHS = 128 * GROUP * WARMG                      # lhsx cols duplicated
WARM_RHS = 128 * (GROUP * WARMG - 1) + 64 + W // 2  # rhsy cols duplicated

_NC_CACHE = {}


def _window_lo(i):
    # y-rank window start for x-rank block i (static, data independent)
    return min(max(128 * i + 64 - W // 2, 0), N - W)


def _build_nc():
    """Build the (per-core SPMD) Bass program. Cached per process.

    Raw Bass (no Tile): the pipeline is PE (banded matmul groups) -> DVE
    (grouped free-axis min reduce) -> SYNC (DMA out), double-buffered over
    two 4-bank PSUM regions with explicit semaphores. Tile's scheduler
    piggybacks >1 sem wait on compute instructions here, which the walrus
    codegen rejects; standalone wait_ge has no such limit.
    """
    if "nc" in _NC_CACHE:
        return _NC_CACHE["nc"]

    import concourse.bass as bass
    import concourse.mybir as mybir

    f32 = mybir.dt.float32
    bf16 = mybir.dt.bfloat16
    nc = bass.Bass()

    # columns: [lhsx | rhsy | lhsy | rhsx], each N wide
    aug_d = nc.dram_tensor("aug", [KAUG, 4 * N], bf16, kind="ExternalInput")
    # warm-start duplicate: operands of the first WARMG groups, small enough
    # to land ~6us in while the 768KB main input is still streaming
    warm_d = nc.dram_tensor("warm", [KAUG, WARM_LHS + WARM_RHS], bf16,
                            kind="ExternalInput")
    out_d = nc.dram_tensor("mins", [128, 2 * NBLK], f32, kind="ExternalOutput")

    NG = 2 * (NBLK // GROUP)  # total reduce groups (both sides)

    with (
        nc.sbuf_tensor("aug_sb", [KAUG, 4 * N], bf16) as aug,
        nc.sbuf_tensor("warm_sb", [KAUG, WARM_LHS + WARM_RHS], bf16) as warm,
        nc.sbuf_tensor("mins_sb", [128, 2 * NBLK], f32) as mins,
        nc.sbuf_tensor("stg_sb", [128, 2 * GROUP * W], bf16) as stg,
        nc.psum_tensor("pt_ps", [128, 2 * GROUP * WPAD], f32) as pt,
        nc.semaphore("dma_sem") as dma_sem,
        nc.semaphore("pe_sem") as pe_sem,
        nc.semaphore("dve_sem") as dve_sem,
        nc.semaphore("act_sem") as act_sem,
        nc.semaphore("ckx") as ckx,
        nc.semaphore("cky") as cky,
        nc.semaphore("ckw") as ckw,
        nc.Block() as block,
    ):
        sb = {
            name: aug[:, k * N : (k + 1) * N]
            for k, name in enumerate(("lhsx", "rhsy", "lhsy", "rhsx"))
        }
        sides = ((sb["lhsx"], sb["rhsy"]), (sb["lhsy"], sb["rhsx"]))

        def group_ap(gi, w):
            # [128, GROUP, w] bank-strided view of the (gi % 2) PSUM region
            base = (gi % 2) * GROUP * WPAD
            full = pt[:, base : base + GROUP * WPAD].rearrange(
                "p (g w) -> p g w", w=WPAD
            )
            return full[:, :, 0:w]

        def stg_ap(gi, w):
            # [128, GROUP, w] view of the (gi % 2) bf16 staging buffer
            base = (gi % 2) * GROUP * W
            full = stg[:, base : base + GROUP * W].rearrange(
                "p (g w) -> p g w", w=W
            )
            return full[:, :, 0:w]

        # groups whose reduce runs directly from PSUM on DVE (to balance
        # ACT-copy vs DVE-tree load); the rest go PSUM -ACT-> bf16 SBUF
        # then a 2x-mode tensor_tensor min tree on DVE
        DIRECT = {6, 14}
        staged = [gi for gi in range(NG) if gi not in DIRECT]
        cpy_idx = {gi: i for i, gi in enumerate(staged)}

        @block.sync
        def _(sync):
            # warm-start chunk first, then x-side (lhsx|rhsy), then y-side
            sync.dma_start(warm[:], warm_d[:]).then_inc(ckw, 16)
            sync.dma_start(aug[:, : 2 * N], aug_d[:, : 2 * N]).then_inc(ckx, 16)
            sync.dma_start(aug[:, 2 * N :], aug_d[:, 2 * N :]).then_inc(cky, 16)
            # first half of the output overlaps side-1 compute
            sync.wait_ge(dve_sem, NG // 2)
            sync.dma_start(out_d[:, :NBLK], mins[:, :NBLK]).then_inc(dma_sem, 16)
            sync.wait_ge(dve_sem, NG)
            sync.dma_start(out_d[:, NBLK:], mins[:, NBLK:]).then_inc(dma_sem, 16)
            sync.wait_ge(dma_sem, 32)

        @block.tensor
        def _(tensor):
            tensor.wait_ge(ckw, 16)
            for gi in range(NG):
                side, g = divmod(gi, NBLK // GROUP)
                if side == 0 and g < WARMG:
                    lhs = warm[:, :WARM_LHS]
                    rhs = warm[:, WARM_LHS:]
                else:
                    lhs, rhs = sides[side]
                if side == 0 and g == WARMG:
                    tensor.wait_ge(ckx, 16)
                if side == 1 and g == 0:
                    tensor.wait_ge(cky, 16)
                if gi >= 2:
                    # WAR: our PSUM region must have been drained by the
                    # consumer of the group two back
                    prev = gi - 2
                    if prev in cpy_idx:
                        tensor.wait_ge(act_sem, cpy_idx[prev] + 1)
                    else:
                        tensor.wait_ge(dve_sem, prev + 1)
                pg = group_ap(gi, W)
                for k in range(GROUP):
                    i = g * GROUP + k
                    lo = _window_lo(i)
                    mm = tensor.matmul(
                        pg[:, k, :],
                        lhs[:, 128 * i : 128 * (i + 1)],
                        rhs[:, lo : lo + W],
                        start=True,
                        stop=True,
                    )
                    if k == GROUP - 1:
                        # MMs complete in pc order; one inc on the last is sound
                        mm.then_inc(pe_sem, 1)

        @block.scalar
        def _(scalar):
            for gi in staged:
                scalar.wait_ge(pe_sem, gi + 1)
                if gi >= 2:
                    # staging buffer reuse: group gi-2's tree must be done
                    scalar.wait_ge(dve_sem, gi - 1)
                scalar.copy(stg_ap(gi, W), group_ap(gi, W)).then_inc(act_sem, 1)

        @block.vector
        def _(vector):
            for gi in range(NG):
                if gi in cpy_idx:
                    vector.wait_ge(act_sem, cpy_idx[gi] + 1)
                    # in-place bf16 tensor_tensor min tree: 352->176->...->11
                    s = stg_ap(gi, W)
                    h = W // 2
                    while h >= 11:
                        vector.tensor_tensor(
                            s[:, :, 0:h], s[:, :, 0:h], s[:, :, h : 2 * h],
                            mybir.AluOpType.min,
                        )
                        h //= 2
                    vector.tensor_reduce(
                        mins[:, gi * GROUP : (gi + 1) * GROUP],
                        s[:, :, 0:11],
                        axis=mybir.AxisListType.X,
                        op=mybir.AluOpType.min,
                    ).then_inc(dve_sem, 1)
                else:
                    vector.wait_ge(pe_sem, gi + 1)
                    vector.tensor_reduce(
                        mins[:, gi * GROUP : (gi + 1) * GROUP],
                        group_ap(gi, W),
                        axis=mybir.AxisListType.X,
                        op=mybir.AluOpType.min,
                    ).then_inc(dve_sem, 1)

    _NC_CACHE["nc"] = nc
    return nc


def _split3(a):
    """Three-level bf16 decomposition: a ~ ah + al + al2 (residual ~2^-27|a|)."""
    import ml_dtypes

    bf = ml_dtypes.bfloat16
    f32 = np.float32
    ah = a.astype(bf).astype(f32)
    r = (a - ah).astype(f32)
    al = r.astype(bf).astype(f32)
    al2 = (r - al).astype(bf).astype(f32)
    return ah, al, al2


def _aug_pair(q, c):
    """bf16-split augmented operands: lhs[:,i] . rhs[:,j] = ||q_i - c_j||^2 / 2.

    All bf16 products are exact in fp32, so accumulating the 6 dominant
    cross terms per coordinate plus triple-split norm rows reproduces the
    fp32 distance to ~1e-7 at bf16 matmul speed (K=24 <= 32 rows is the
    same PE cost as K=5).
    """
    f32 = np.float32
    lhs_rows, rhs_rows = [], []
    for d in range(D):
        ah, al, al2 = _split3(q[:, d])
        bh, bl, bl2 = _split3(-c[:, d])
        lhs_rows += [ah, ah, al, al, ah, al2]
        rhs_rows += [bh, bl, bh, bl, bl2, bh]
    qd = 0.5 * (q * q).sum(1, dtype=np.float64)
    cd = 0.5 * (c * c).sum(1, dtype=np.float64)
    ones = np.ones(N, f32)
    qh, ql, ql2 = _split3(qd.astype(f32))
    ch, cl, cl2 = _split3(cd.astype(f32))
    lhs_rows += [qh, ql, ql2, ones, ones, ones]
    rhs_rows += [ones, ones, ones, ch, cl, cl2]
    import ml_dtypes

    return (
        np.stack(lhs_rows).astype(ml_dtypes.bfloat16),
        np.stack(rhs_rows).astype(ml_dtypes.bfloat16),
    )


def _prep_batch(x, y):
    """Sort by coord 0 and build the augmented matmul operands (host side)."""
    xs = x[np.argsort(x[:, 0], kind="stable")]
    ys = y[np.argsort(y[:, 0], kind="stable")]

    lhsx, rhsy = _aug_pair(xs, ys)
    lhsy, rhsx = _aug_pair(ys, xs)
    aug = np.concatenate([lhsx, rhsy, lhsy, rhsx], axis=1)
    warm = np.concatenate([lhsx[:, :WARM_LHS], rhsy[:, :WARM_RHS]], axis=1)
    return xs, ys, {
        "aug": np.ascontiguousarray(aug),
        "warm": np.ascontiguousarray(warm),
    }


def _fix_side(mins, qs, cs):
    """Posterior exactness check + exact host fixup for unproven rows.

    mins: device banded row minima (full P scale) for sorted queries qs
    against sorted candidates cs. Returns exact per-row minima.
    """
    i = np.arange(N) // 128
    lo = np.clip(128 * i + 64 - W // 2, 0, N - W)
    hi = lo + W
    lb = np.full(N, np.inf)
    has_l = lo > 0
    lb[has_l] = np.maximum(0.0, qs[has_l, 0] - cs[lo[has_l] - 1, 0]) ** 2
    has_r = hi < N
    lb[has_r] = np.minimum(
        lb[has_r], np.maximum(0.0, cs[np.minimum(hi[has_r], N - 1), 0] - qs[has_r, 0]) ** 2
    )
    unproven = mins > lb - 1e-5
    if unproven.any():
        rows = np.where(unproven)[0]
        d = qs[rows, None, :].astype(np.float64) - cs[None, :, :].astype(np.float64)
        exact = (d * d).sum(-1).min(1)
        out = mins.copy()
        out[rows] = np.minimum(mins[rows], exact.astype(np.float32))
        return out
    return mins


def _postprocess(results, meta):
    """Combine per-core device outputs into the final scalar."""
    total = 0.0
    for b in range(B):
        xs, ys = meta[b]
        m = results[b]["mins"]  # [128, 2*NBLK]; [p, s*NBLK+i] = min for rank 128*i+p
        mx = 2.0 * np.ascontiguousarray(m[:, :NBLK].T).reshape(N)  # x queries vs y
        my = 2.0 * np.ascontiguousarray(m[:, NBLK:].T).reshape(N)  # y queries vs x
        mx = _fix_side(mx, xs, ys)
        my = _fix_side(my, ys, xs)
        total += mx.mean(dtype=np.float64) + my.mean(dtype=np.float64)
    return np.array(total / B, dtype=np.float32)


def _run(inputs, trace=False):
    p1 = np.ascontiguousarray(np.asarray(inputs["p1"], dtype=np.float32))
    p2 = np.ascontiguousarray(np.asarray(inputs["p2"], dtype=np.float32))
    assert p1.shape == (B, N, D) and p2.shape == (B, N, D)

    in_maps = []
    meta = []
    for b in range(B):
        xs, ys, im = _prep_batch(p1[b], p2[b])
        in_maps.append(im)
        meta.append((xs, ys))

    from concourse.bass_utils import run_bass_kernel_spmd

    nc = _build_nc()
    kw = {}
    if trace:
        kw = dict(trace=True, trace_cores=list(range(N_CORES)))
    res = run_bass_kernel_spmd(nc, in_maps, list(range(N_CORES)), **kw)
    return _postprocess(res.results, meta), res


def kernel(**inputs):
    out, _ = _run(inputs, trace=False)
    return out


def kernel_traced(**inputs):
    """Same as kernel() but also returns BassKernelResults with NTFF timing."""
    return _run(inputs, trace=True)


# revision 39
# speedup vs baseline: 1.0539x; 1.0539x over previous
"""Chamfer distance kernel for Trainium2 (8 NeuronCores, Bass/Tile).

Problem: p1, p2 are [B=8, N=4096, D=3] fp32 point clouds. Output is the
scalar  mean_j(min_i P[b,i,j]) + mean_i(min_j P[b,i,j])  where
P[b,i,j] = ||p1[b,i] - p2[b,j]||^2.

Strategy
--------
Data-parallel over B: core b handles batch b.

Nearest-neighbor structure: on the host each batch's points are sorted by
coordinate 0. Nearest neighbors are then close in *rank*, so instead of the
full [N, N] distance matrix each 128-point block only scans a W-wide window
of rank-adjacent candidates (a banded distance matrix). Both directions
(min over rows / min over cols) are computed as separate banded passes with
the roles of the two point sets swapped, so on-device both reductions are
free-axis `tensor_reduce(min)` ops.

Each distance block is a single matmul via the augmentation
  lhsT rows ~ [x0, x1, x2, ||x||^2/2, 1]
  rhs  rows ~ [-y0, -y1, -y2, 1, ||y||^2/2]
giving P/2 per element; row mins are doubled on the host. The fp32 rows are
triple-split into bf16 components (K=24, see _aug_pair) because the PE runs
fp32 matmuls at 1/4 rate while K<=32 bf16 costs the same as K=5 — this keeps
fp32-level accuracy (~1e-7) at full bf16 speed.

Exactness: banding alone can miss isolated points. For each row the host
runs an O(1) posterior bound check — every candidate outside the window has
dist^2 >= (coord0 gap to the window edge)^2, so a row whose banded min is
below that gap is *provably* exact. The few unproven rows (~1.5% at W=352 on
randn data) are recomputed exactly on the host with a full scan.
"""

import sys

import numpy as np

if "/opt/trn_rl_repo" not in sys.path:
    sys.path.insert(0, "/opt/trn_rl_repo")

B = 8
N = 4096
D = 3
W = 352          # band width (candidates per 128-row block)
WPAD = 512       # PSUM bank stride per block (fp32 elems; 2KB bank)
NBLK = N // 128  # 32 row blocks per side
GROUP = 4        # blocks reduced per tensor_reduce (4 PSUM banks)
N_CORES = 8
KAUG = 24        # bf16-split augmented contraction dim (see _aug_pair)
WARMG = 4        # leading x-side groups served from the warm-start chunk
WARM_LHS = 128 * GROUP * WARMG                      # lhsx cols duplicated
WARM_RHS = 128 * (GROUP * WARMG - 1) + 64 + W // 2  # rhsy cols duplicated

_NC_CACHE = {}


def _window_lo(i):
    # y-rank window start for x-rank block i (static, data independent)
    return min(max(128 * i + 64 - W // 2, 0), N - W)


def _build_nc():
    """Build the (per-core SPMD) Bass program. Cached per process.

    Raw Bass (no Tile): the pipeline is PE (banded matmul groups) -> DVE
    (grouped free-axis min reduce) -> SYNC (DMA out), double-buffered over
    two 4-bank PSUM regions with explicit semaphores. Tile's scheduler
    piggybacks >1 sem wait on compute instructions here, which the walrus
    codegen rejects; standalone wait_ge has no such limit.
    """
    if "nc" in _NC_CACHE:
        return _NC_CACHE["nc"]

    import concourse.bass as bass
    import concourse.mybir as mybir

    f32 = mybir.dt.float32
    bf16 = mybir.dt.bfloat16
    nc = bass.Bass()

    # columns: [lhsx | rhsy | lhsy | rhsx], each N wide
    aug_d = nc.dram_tensor("aug", [KAUG, 4 * N], bf16, kind="ExternalInput")
    # warm-start duplicate: operands of the first WARMG groups, small enough
    # to land ~6us in while the 768KB main input is still streaming
    warm_d = nc.dram_tensor("warm", [KAUG, WARM_LHS + WARM_RHS], bf16,
                            kind="ExternalInput")
    out_d = nc.dram_tensor("mins", [128, 2 * NBLK], f32, kind="ExternalOutput")

    NG = 2 * (NBLK // GROUP)  # total reduce groups (both sides)

    with (
        nc.sbuf_tensor("aug_sb", [KAUG, 4 * N], bf16) as aug,
        nc.sbuf_tensor("warm_sb", [KAUG, WARM_LHS + WARM_RHS], bf16) as warm,
        nc.sbuf_tensor("mins_sb", [128, 2 * NBLK], f32) as mins,
        nc.psum_tensor("pt_ps", [128, 2 * GROUP * WPAD], f32) as pt,
        nc.semaphore("dma_sem") as dma_sem,
        nc.semaphore("pe_sem") as pe_sem,
        nc.semaphore("dve_sem") as dve_sem,
        nc.semaphore("ckx") as ckx,
        nc.semaphore("cky") as cky,
        nc.semaphore("ckw") as ckw,
        nc.Block() as block,
    ):
        sb = {
            name: aug[:, k * N : (k + 1) * N]
            for k, name in enumerate(("lhsx", "rhsy", "lhsy", "rhsx"))
        }
        sides = ((sb["lhsx"], sb["rhsy"]), (sb["lhsy"], sb["rhsx"]))

        def group_ap(gi, w):
            # [128, GROUP, w] bank-strided view of the (gi % 2) PSUM region
            base = (gi % 2) * GROUP * WPAD
            full = pt[:, base : base + GROUP * WPAD].rearrange(
                "p (g w) -> p g w", w=WPAD
            )
            return full[:, :, 0:w]

        @block.sync
        def _(sync):
            # warm-start chunk first, then x-side (lhsx|rhsy), then y-side
            sync.dma_start(warm[:], warm_d[:]).then_inc(ckw, 16)
            sync.dma_start(aug[:, : 2 * N], aug_d[:, : 2 * N]).then_inc(ckx, 16)
            sync.dma_start(aug[:, 2 * N :], aug_d[:, 2 * N :]).then_inc(cky, 16)
            # first half of the output overlaps side-1 compute
            sync.wait_ge(dve_sem, NG // 2)
            sync.dma_start(out_d[:, :NBLK], mins[:, :NBLK]).then_inc(dma_sem, 16)
            sync.wait_ge(dve_sem, NG)
            sync.dma_start(out_d[:, NBLK:], mins[:, NBLK:]).then_inc(dma_sem, 16)
            sync.wait_ge(dma_sem, 32)

        @block.tensor
        def _(tensor):
            tensor.wait_ge(ckw, 16)
            for gi in range(NG):
                side, g = divmod(gi, NBLK // GROUP)
                if side == 0 and g < WARMG:
                    lhs = warm[:, :WARM_LHS]
                    rhs = warm[:, WARM_LHS:]
                else:
                    lhs, rhs = sides[side]
                if side == 0 and g == WARMG:
                    tensor.wait_ge(ckx, 16)
                if side == 1 and g == 0:
                    tensor.wait_ge(cky, 16)
                if gi >= 2:
                    # WAR: our PSUM region must have been drained by the
                    # reduce two groups back
                    tensor.wait_ge(dve_sem, gi - 1)
                pg = group_ap(gi, W)
                for k in range(GROUP):
                    i = g * GROUP + k
                    lo = _window_lo(i)
                    mm = tensor.matmul(
                        pg[:, k, :],
                        lhs[:, 128 * i : 128 * (i + 1)],
                        rhs[:, lo : lo + W],
                        start=True,
                        stop=True,
                    )
                    if k == GROUP - 1:
                        # MMs complete in pc order; one inc on the last is sound
                        mm.then_inc(pe_sem, 1)

        @block.vector
        def _(vector):
            for gi in range(NG):
                vector.wait_ge(pe_sem, gi + 1)
                vector.tensor_reduce(
                    mins[:, gi * GROUP : (gi + 1) * GROUP],
                    group_ap(gi, W),
                    axis=mybir.AxisListType.X,
                    op=mybir.AluOpType.min,
                ).then_inc(dve_sem, 1)

    _NC_CACHE["nc"] = nc
    return nc


def _split3(a):
    """Three-level bf16 decomposition: a ~ ah + al + al2 (residual ~2^-27|a|)."""
    import ml_dtypes

    bf = ml_dtypes.bfloat16
    f32 = np.float32
    ah = a.astype(bf).astype(f32)
    r = (a - ah).astype(f32)
    al = r.astype(bf).astype(f32)
    al2 = (r - al).astype(bf).astype(f32)
    return ah, al, al2


def _aug_pair(q, c):
    """bf16-split augmented operands: lhs[:,i] . rhs[:,j] = ||q_i - c_j||^2 / 2.

    All bf16 products are exact in fp32, so accumulating the 6 dominant
    cross terms per coordinate plus triple-split norm rows reproduces the
    fp32 distance to ~1e-7 at bf16 matmul speed (K=24 <= 32 rows is the
    same PE cost as K=5).
    """
    f32 = np.float32
    lhs_rows, rhs_rows = [], []
    for d in range(D):
        ah, al, al2 = _split3(q[:, d])
        bh, bl, bl2 = _split3(-c[:, d])
        lhs_rows += [ah, ah, al, al, ah, al2]
        rhs_rows += [bh, bl, bh, bl, bl2, bh]
    qd = 0.5 * (q * q).sum(1, dtype=np.float64)
    cd = 0.5 * (c * c).sum(1, dtype=np.float64)
    ones = np.ones(N, f32)
    qh, ql, ql2 = _split3(qd.astype(f32))
    ch, cl, cl2 = _split3(cd.astype(f32))
    lhs_rows += [qh, ql, ql2, ones, ones, ones]
    rhs_rows += [ones, ones, ones, ch, cl, cl2]
    import ml_dtypes

    return (
        np.stack(lhs_rows).astype(ml_dtypes.bfloat16),
        np.stack(rhs_rows).astype(ml_dtypes.bfloat16),
    )


def _prep_batch(x, y):
    """Sort by coord 0 and build the augmented matmul operands (host side)."""
    xs = x[np.argsort(x[:, 0], kind="stable")]
    ys = y[np.argsort(y[:, 0], kind="stable")]

    lhsx, rhsy = _aug_pair(xs, ys)
    lhsy, rhsx = _aug_pair(ys, xs)
    aug = np.concatenate([lhsx, rhsy, lhsy, rhsx], axis=1)
    warm = np.concatenate([lhsx[:, :WARM_LHS], rhsy[:, :WARM_RHS]], axis=1)
    return xs, ys, {
        "aug": np.ascontiguousarray(aug),
        "warm": np.ascontiguousarray(warm),
    }


def _fix_side(mins, qs, cs):
    """Posterior exactness check + exact host fixup for unproven rows.

    mins: device banded row minima (full P scale) for sorted queries qs
    against sorted candidates cs. Returns exact per-row minima.
    """
    i = np.arange(N) // 128
    lo = np.clip(128 * i + 64 - W // 2, 0, N - W)
    hi = lo + W
    lb = np.full(N, np.inf)
    has_l = lo > 0
    lb[has_l] = np.maximum(0.0, qs[has_l, 0] - cs[lo[has_l] - 1, 0]) ** 2
    has_r = hi < N
    lb[has_r] = np.minimum(
        lb[has_r], np.maximum(0.0, cs[np.minimum(hi[has_r], N - 1), 0] - qs[has_r, 0]) ** 2
    )
    unproven = mins > lb - 1e-5
    if unproven.any():
        rows = np.where(unproven)[0]
        d = qs[rows, None, :].astype(np.float64) - cs[None, :, :].astype(np.float64)
        exact = (d * d).sum(-1).min(1)
        out = mins.copy()
        out[rows] = np.minimum(mins[rows], exact.astype(np.float32))
        return out
    return mins


def _postprocess(results, meta):
    """Combine per-core device outputs into the final scalar."""
    total = 0.0
    for b in range(B):
        xs, ys = meta[b]
        m = results[b]["mins"]  # [128, 2*NBLK]; [p, s*NBLK+i] = min for rank 128*i+p
        mx = 2.0 * np.ascontiguousarray(m[:, :NBLK].T).reshape(N)  # x queries vs y
        my = 2.0 * np.ascontiguousarray(m[:, NBLK:].T).reshape(N)  # y queries vs x
        mx = _fix_side(mx, xs, ys)
        my = _fix_side(my, ys, xs)
        total += mx.mean(dtype=np.float64) + my.mean(dtype=np.float64)
    return np.array(total / B, dtype=np.float32)


def _run(inputs, trace=False):
    p1 = np.ascontiguousarray(np.asarray(inputs["p1"], dtype=np.float32))
    p2 = np.ascontiguousarray(np.asarray(inputs["p2"], dtype=np.float32))
    assert p1.shape == (B, N, D) and p2.shape == (B, N, D)

    in_maps = []
    meta = []
    for b in range(B):
        xs, ys, im = _prep_batch(p1[b], p2[b])
        in_maps.append(im)
        meta.append((xs, ys))

    from concourse.bass_utils import run_bass_kernel_spmd

    nc = _build_nc()
    kw = {}
    if trace:
        kw = dict(trace=True, trace_cores=list(range(N_CORES)))
    res = run_bass_kernel_spmd(nc, in_maps, list(range(N_CORES)), **kw)
    return _postprocess(res.results, meta), res


def kernel(**inputs):
    out, _ = _run(inputs, trace=False)
    return out


def kernel_traced(**inputs):
    """Same as kernel() but also returns BassKernelResults with NTFF timing."""
    return _run(inputs, trace=True)


# revision 47
# speedup vs baseline: 1.0565x; 1.0025x over previous
"""Chamfer distance kernel for Trainium2 (8 NeuronCores, Bass/Tile).

Problem: p1, p2 are [B=8, N=4096, D=3] fp32 point clouds. Output is the
scalar  mean_j(min_i P[b,i,j]) + mean_i(min_j P[b,i,j])  where
P[b,i,j] = ||p1[b,i] - p2[b,j]||^2.

Strategy
--------
Data-parallel over B: core b handles batch b.

Nearest-neighbor structure: on the host each batch's points are sorted by
coordinate 0. Nearest neighbors are then close in *rank*, so instead of the
full [N, N] distance matrix each 128-point block only scans a W-wide window
of rank-adjacent candidates (a banded distance matrix). Both directions
(min over rows / min over cols) are computed as separate banded passes with
the roles of the two point sets swapped, so on-device both reductions are
free-axis `tensor_reduce(min)` ops.

Each distance block is a single matmul via the augmentation
  lhsT rows ~ [x0, x1, x2, ||x||^2/2, 1]
  rhs  rows ~ [-y0, -y1, -y2, 1, ||y||^2/2]
giving P/2 per element; row mins are doubled on the host. The fp32 rows are
triple-split into bf16 components (K=24, see _aug_pair) because the PE runs
fp32 matmuls at 1/4 rate while K<=32 bf16 costs the same as K=5 — this keeps
fp32-level accuracy (~1e-7) at full bf16 speed.

Exactness: banding alone can miss isolated points. For each row the host
runs an O(1) posterior bound check — every candidate outside the window has
dist^2 >= (coord0 gap to the window edge)^2, so a row whose banded min is
below that gap is *provably* exact. The few unproven rows (~1.5% at W=352 on
randn data) are recomputed exactly on the host with a full scan.
"""

import sys

import numpy as np

if "/opt/trn_rl_repo" not in sys.path:
    sys.path.insert(0, "/opt/trn_rl_repo")

B = 8
N = 4096
D = 3
W = 320          # band width (candidates per 128-row block)
WPAD = 512       # PSUM bank stride per block (fp32 elems; 2KB bank)
NBLK = N // 128  # 32 row blocks per side
GROUP = 4        # blocks reduced per tensor_reduce (4 PSUM banks)
N_CORES = 8
KAUG = 24        # bf16-split augmented contraction dim (see _aug_pair)
WARMG = 4        # leading x-side groups served from the warm-start chunk
WARM_LHS = 128 * GROUP * WARMG                      # lhsx cols duplicated
WARM_RHS = 128 * (GROUP * WARMG - 1) + 64 + W // 2  # rhsy cols duplicated
# reduce strategy: groups in DIRECT tensor_reduce straight from PSUM on DVE;
# the rest are drained PSUM->bf16 SBUF by the otherwise-idle ScalarE, then
# min-reduced on DVE (tree at 2x TT mode if USE_TREE, else plain reduce)
DIRECT = frozenset()
USE_TREE = False

_NC_CACHE = {}


def _window_lo(i):
    # y-rank window start for x-rank block i (static, data independent)
    return min(max(128 * i + 64 - W // 2, 0), N - W)


def _build_nc():
    """Build the (per-core SPMD) Bass program. Cached per process.

    Raw Bass (no Tile): the pipeline is PE (banded matmul groups) -> DVE
    (grouped free-axis min reduce) -> SYNC (DMA out), double-buffered over
    two 4-bank PSUM regions with explicit semaphores. Tile's scheduler
    piggybacks >1 sem wait on compute instructions here, which the walrus
    codegen rejects; standalone wait_ge has no such limit.
    """
    if "nc" in _NC_CACHE:
        return _NC_CACHE["nc"]

    import concourse.bass as bass
    import concourse.mybir as mybir

    f32 = mybir.dt.float32
    bf16 = mybir.dt.bfloat16
    nc = bass.Bass()

    # columns: [lhsx | rhsy | lhsy | rhsx], each N wide
    aug_d = nc.dram_tensor("aug", [KAUG, 4 * N], bf16, kind="ExternalInput")
    # warm-start duplicate: operands of the first WARMG groups, small enough
    # to land ~6us in while the 768KB main input is still streaming
    warm_d = nc.dram_tensor("warm", [KAUG, WARM_LHS + WARM_RHS], bf16,
                            kind="ExternalInput")
    out_d = nc.dram_tensor("mins", [128, 2 * NBLK], f32, kind="ExternalOutput")

    NG = 2 * (NBLK // GROUP)  # total reduce groups (both sides)

    with (
        nc.sbuf_tensor("aug_sb", [KAUG, 4 * N], bf16) as aug,
        nc.sbuf_tensor("warm_sb", [KAUG, WARM_LHS + WARM_RHS], bf16) as warm,
        nc.sbuf_tensor("mins_sb", [128, 2 * NBLK], f32) as mins,
        nc.sbuf_tensor("stg_sb", [128, 2 * GROUP * W], f32) as stg,
        nc.sbuf_tensor("sc_sb", [128, GROUP * W], f32) as sc,
        nc.psum_tensor("pt_ps", [128, 2 * GROUP * WPAD], f32) as pt,
        nc.semaphore("dma_sem") as dma_sem,
        nc.semaphore("pe_sem") as pe_sem,
        nc.semaphore("dve_sem") as dve_sem,
        nc.semaphore("act_sem") as act_sem,
        nc.semaphore("ckx") as ckx,
        nc.semaphore("cky") as cky,
        nc.semaphore("ckw") as ckw,
        nc.Block() as block,
    ):
        sb = {
            name: aug[:, k * N : (k + 1) * N]
            for k, name in enumerate(("lhsx", "rhsy", "lhsy", "rhsx"))
        }
        sides = ((sb["lhsx"], sb["rhsy"]), (sb["lhsy"], sb["rhsx"]))

        def group_ap(gi, w):
            # [128, GROUP, w] bank-strided view of the (gi % 2) PSUM region
            base = (gi % 2) * GROUP * WPAD
            full = pt[:, base : base + GROUP * WPAD].rearrange(
                "p (g w) -> p g w", w=WPAD
            )
            return full[:, :, 0:w]

        def stg_ap(gi, w):
            # [128, GROUP, w] view of the (gi % 2) bf16 staging buffer
            base = (gi % 2) * GROUP * W
            full = stg[:, base : base + GROUP * W].rearrange(
                "p (g w) -> p g w", w=W
            )
            return full[:, :, 0:w]

        def sc_ap(half, w):
            # ping-pong scratch [128, GROUP, w] (half 0: cols 0:W/2 of each
            # group slot; half 1: cols W/2:W) — DVE-serial use, no sems
            full = sc[:].rearrange("p (g w) -> p g w", w=W)
            return full[:, :, half * (W // 2) : half * (W // 2) + w]

        staged = [gi for gi in range(NG) if gi not in DIRECT]
        cpy_idx = {gi: i for i, gi in enumerate(staged)}

        @block.sync
        def _(sync):
            # warm-start chunk first, then x-side (lhsx|rhsy), then y-side
            sync.dma_start(warm[:], warm_d[:]).then_inc(ckw, 16)
            sync.dma_start(aug[:, : 2 * N], aug_d[:, : 2 * N]).then_inc(ckx, 16)
            sync.dma_start(aug[:, 2 * N :], aug_d[:, 2 * N :]).then_inc(cky, 16)
            # first half of the output overlaps side-1 compute
            sync.wait_ge(dve_sem, NG // 2)
            sync.dma_start(out_d[:, :NBLK], mins[:, :NBLK]).then_inc(dma_sem, 16)
            sync.wait_ge(dve_sem, NG)
            sync.dma_start(out_d[:, NBLK:], mins[:, NBLK:]).then_inc(dma_sem, 16)
            sync.wait_ge(dma_sem, 32)

        @block.tensor
        def _(tensor):
            tensor.wait_ge(ckw, 16)
            for gi in range(NG):
                side, g = divmod(gi, NBLK // GROUP)
                if side == 0 and g < WARMG:
                    lhs = warm[:, :WARM_LHS]
                    rhs = warm[:, WARM_LHS:]
                else:
                    lhs, rhs = sides[side]
                if side == 0 and g == WARMG:
                    tensor.wait_ge(ckx, 16)
                if side == 1 and g == 0:
                    tensor.wait_ge(cky, 16)
                if gi >= 2:
                    # WAR: our PSUM region must have been drained by the
                    # consumer of the group two back
                    prev = gi - 2
                    if prev in cpy_idx:
                        tensor.wait_ge(act_sem, cpy_idx[prev] + 1)
                    else:
                        tensor.wait_ge(dve_sem, prev + 1)
                pg = group_ap(gi, W)
                for k in range(GROUP):
                    i = g * GROUP + k
                    lo = _window_lo(i)
                    mm = tensor.matmul(
                        pg[:, k, :],
                        lhs[:, 128 * i : 128 * (i + 1)],
                        rhs[:, lo : lo + W],
                        start=True,
                        stop=True,
                    )
                    if k == GROUP - 1:
                        # MMs complete in pc order; one inc on the last is sound
                        mm.then_inc(pe_sem, 1)

        @block.scalar
        def _(scalar):
            # dummy copy: absorbs the one-time ACT table load (~2.7us)
            # while the input DMA is still in flight
            scalar.copy(sc[0:1, 0:8], sc[0:1, 8:16])
            for gi in staged:
                scalar.wait_ge(pe_sem, gi + 1)
                if gi >= 2:
                    # staging buffer reuse: group gi-2's reduce must be done
                    scalar.wait_ge(dve_sem, gi - 1)
                scalar.copy(stg_ap(gi, W), group_ap(gi, W)).then_inc(act_sem, 1)

        @block.vector
        def _(vector):
            for gi in range(NG):
                out_ap = mins[:, gi * GROUP : (gi + 1) * GROUP]
                if gi in cpy_idx:
                    vector.wait_ge(act_sem, cpy_idx[gi] + 1)
                    if USE_TREE:
                        # bf16 2x-mode TT min tree, ping-pong scratch halves
                        # (DVE-serial: no sync needed between levels)
                        src, w, half = stg_ap(gi, W), W // 2, 0
                        while w >= 11:
                            dst = sc_ap(half, w)
                            vector.tensor_tensor(
                                dst, src[:, :, 0:w], src[:, :, w : 2 * w],
                                mybir.AluOpType.min,
                            )
                            src, w, half = dst, w // 2, 1 - half
                        vector.tensor_reduce(
                            out_ap, src, axis=mybir.AxisListType.X,
                            op=mybir.AluOpType.min,
                        ).then_inc(dve_sem, 1)
                    else:
                        vector.tensor_reduce(
                            out_ap, stg_ap(gi, W), axis=mybir.AxisListType.X,
                            op=mybir.AluOpType.min,
                        ).then_inc(dve_sem, 1)
                else:
                    vector.wait_ge(pe_sem, gi + 1)
                    vector.tensor_reduce(
                        out_ap, group_ap(gi, W), axis=mybir.AxisListType.X,
                        op=mybir.AluOpType.min,
                    ).then_inc(dve_sem, 1)

    _NC_CACHE["nc"] = nc
    return nc


def _split3(a):
    """Three-level bf16 decomposition: a ~ ah + al + al2 (residual ~2^-27|a|)."""
    import ml_dtypes

    bf = ml_dtypes.bfloat16
    f32 = np.float32
    ah = a.astype(bf).astype(f32)
    r = (a - ah).astype(f32)
    al = r.astype(bf).astype(f32)
    al2 = (r - al).astype(bf).astype(f32)
    return ah, al, al2


def _aug_pair(q, c):
    """bf16-split augmented operands: lhs[:,i] . rhs[:,j] = ||q_i - c_j||^2 / 2.

    All bf16 products are exact in fp32, so accumulating the 6 dominant
    cross terms per coordinate plus triple-split norm rows reproduces the
    fp32 distance to ~1e-7 at bf16 matmul speed (K=24 <= 32 rows is the
    same PE cost as K=5).
    """
    f32 = np.float32
    lhs_rows, rhs_rows = [], []
    for d in range(D):
        ah, al, al2 = _split3(q[:, d])
        bh, bl, bl2 = _split3(-c[:, d])
        lhs_rows += [ah, ah, al, al, ah, al2]
        rhs_rows += [bh, bl, bh, bl, bl2, bh]
    qd = 0.5 * (q * q).sum(1, dtype=np.float64)
    cd = 0.5 * (c * c).sum(1, dtype=np.float64)
    ones = np.ones(N, f32)
    qh, ql, ql2 = _split3(qd.astype(f32))
    ch, cl, cl2 = _split3(cd.astype(f32))
    lhs_rows += [qh, ql, ql2, ones, ones, ones]
    rhs_rows += [ones, ones, ones, ch, cl, cl2]
    import ml_dtypes

    return (
        np.stack(lhs_rows).astype(ml_dtypes.bfloat16),
        np.stack(rhs_rows).astype(ml_dtypes.bfloat16),
    )


def _prep_batch(x, y):
    """Sort by coord 0 and build the augmented matmul operands (host side)."""
    xs = x[np.argsort(x[:, 0], kind="stable")]
    ys = y[np.argsort(y[:, 0], kind="stable")]

    lhsx, rhsy = _aug_pair(xs, ys)
    lhsy, rhsx = _aug_pair(ys, xs)
    aug = np.concatenate([lhsx, rhsy, lhsy, rhsx], axis=1)
    warm = np.concatenate([lhsx[:, :WARM_LHS], rhsy[:, :WARM_RHS]], axis=1)
    return xs, ys, {
        "aug": np.ascontiguousarray(aug),
        "warm": np.ascontiguousarray(warm),
    }


def _fix_side(mins, qs, cs):
    """Posterior exactness check + exact host fixup for unproven rows.

    mins: device banded row minima (full P scale) for sorted queries qs
    against sorted candidates cs. Returns exact per-row minima.
    """
    i = np.arange(N) // 128
    lo = np.clip(128 * i + 64 - W // 2, 0, N - W)
    hi = lo + W
    lb = np.full(N, np.inf)
    has_l = lo > 0
    lb[has_l] = np.maximum(0.0, qs[has_l, 0] - cs[lo[has_l] - 1, 0]) ** 2
    has_r = hi < N
    lb[has_r] = np.minimum(
        lb[has_r], np.maximum(0.0, cs[np.minimum(hi[has_r], N - 1), 0] - qs[has_r, 0]) ** 2
    )
    unproven = mins > lb - 1e-5
    if unproven.any():
        rows = np.where(unproven)[0]
        d = qs[rows, None, :].astype(np.float64) - cs[None, :, :].astype(np.float64)
        exact = (d * d).sum(-1).min(1)
        out = mins.copy()
        out[rows] = np.minimum(mins[rows], exact.astype(np.float32))
        return out
    return mins


def _postprocess(results, meta):
    """Combine per-core device outputs into the final scalar."""
    total = 0.0
    for b in range(B):
        xs, ys = meta[b]
        m = results[b]["mins"]  # [128, 2*NBLK]; [p, s*NBLK+i] = min for rank 128*i+p
        mx = 2.0 * np.ascontiguousarray(m[:, :NBLK].T).reshape(N)  # x queries vs y
        my = 2.0 * np.ascontiguousarray(m[:, NBLK:].T).reshape(N)  # y queries vs x
        mx = _fix_side(mx, xs, ys)
        my = _fix_side(my, ys, xs)
        total += mx.mean(dtype=np.float64) + my.mean(dtype=np.float64)
    return np.array(total / B, dtype=np.float32)


def _run(inputs, trace=False):
    p1 = np.ascontiguousarray(np.asarray(inputs["p1"], dtype=np.float32))
    p2 = np.ascontiguousarray(np.asarray(inputs["p2"], dtype=np.float32))
    assert p1.shape == (B, N, D) and p2.shape == (B, N, D)

    in_maps = []
    meta = []
    for b in range(B):
        xs, ys, im = _prep_batch(p1[b], p2[b])
        in_maps.append(im)
        meta.append((xs, ys))

    from concourse.bass_utils import run_bass_kernel_spmd

    nc = _build_nc()
    kw = {}
    if trace:
        kw = dict(trace=True, trace_cores=list(range(N_CORES)))
    res = run_bass_kernel_spmd(nc, in_maps, list(range(N_CORES)), **kw)
    return _postprocess(res.results, meta), res


def kernel(**inputs):
    out, _ = _run(inputs, trace=False)
    return out


def kernel_traced(**inputs):
    """Same as kernel() but also returns BassKernelResults with NTFF timing."""
    return _run(inputs, trace=True)


# revision 48
# speedup vs baseline: 1.1078x; 1.0486x over previous
"""Chamfer distance kernel for Trainium2 (8 NeuronCores, Bass/Tile).

Problem: p1, p2 are [B=8, N=4096, D=3] fp32 point clouds. Output is the
scalar  mean_j(min_i P[b,i,j]) + mean_i(min_j P[b,i,j])  where
P[b,i,j] = ||p1[b,i] - p2[b,j]||^2.

Strategy
--------
Data-parallel over B: core b handles batch b.

Nearest-neighbor structure: on the host each batch's points are sorted by
coordinate 0. Nearest neighbors are then close in *rank*, so instead of the
full [N, N] distance matrix each 128-point block only scans a W-wide window
of rank-adjacent candidates (a banded distance matrix). Both directions
(min over rows / min over cols) are computed as separate banded passes with
the roles of the two point sets swapped, so on-device both reductions are
free-axis `tensor_reduce(min)` ops.

Each distance block is a single matmul via the augmentation
  lhsT rows ~ [x0, x1, x2, ||x||^2/2, 1]
  rhs  rows ~ [-y0, -y1, -y2, 1, ||y||^2/2]
giving P/2 per element; row mins are doubled on the host. The fp32 rows are
triple-split into bf16 components (K=24, see _aug_pair) because the PE runs
fp32 matmuls at 1/4 rate while K<=32 bf16 costs the same as K=5 — this keeps
fp32-level accuracy (~1e-7) at full bf16 speed.

Exactness: banding alone can miss isolated points. For each row the host
runs an O(1) posterior bound check — every candidate outside the window has
dist^2 >= (coord0 gap to the window edge)^2, so a row whose banded min is
below that gap is *provably* exact. The few unproven rows (~1.5% at W=352 on
randn data) are recomputed exactly on the host with a full scan.
"""

import sys

import numpy as np

if "/opt/trn_rl_repo" not in sys.path:
    sys.path.insert(0, "/opt/trn_rl_repo")

B = 8
N = 4096
D = 3
W = 320          # band width (candidates per 128-row block)
WPAD = 512       # PSUM bank stride per block (fp32 elems; 2KB bank)
NBLK = N // 128  # 32 row blocks per side
GROUP = 4        # blocks reduced per tensor_reduce (4 PSUM banks)
N_CORES = 8
KAUG = 24        # bf16-split augmented contraction dim (see _aug_pair)
WARMG = 4        # leading x-side groups served from the warm-start chunk
WARM_LHS = 128 * GROUP * WARMG                      # lhsx cols duplicated
WARM_RHS = 128 * (GROUP * WARMG - 1) + 64 + W // 2  # rhsy cols duplicated
# reduce strategy: groups in DIRECT tensor_reduce straight from PSUM on DVE;
# the rest are drained PSUM->bf16 SBUF by the otherwise-idle ScalarE, then
# min-reduced on DVE (tree at 2x TT mode if USE_TREE, else plain reduce)
DIRECT = frozenset(range(16))  # staging via ScalarE measured no faster:
USE_TREE = False               # the strided PSUM copy pays the 172cyc
                               # read-write bubble per bank, matching the
                               # direct reduce; keep the simple path

_NC_CACHE = {}


def _window_lo(i):
    # y-rank window start for x-rank block i (static, data independent)
    return min(max(128 * i + 64 - W // 2, 0), N - W)


def _build_nc():
    """Build the (per-core SPMD) Bass program. Cached per process.

    Raw Bass (no Tile): the pipeline is PE (banded matmul groups) -> DVE
    (grouped free-axis min reduce) -> SYNC (DMA out), double-buffered over
    two 4-bank PSUM regions with explicit semaphores. Tile's scheduler
    piggybacks >1 sem wait on compute instructions here, which the walrus
    codegen rejects; standalone wait_ge has no such limit.
    """
    if "nc" in _NC_CACHE:
        return _NC_CACHE["nc"]

    import concourse.bass as bass
    import concourse.mybir as mybir

    f32 = mybir.dt.float32
    bf16 = mybir.dt.bfloat16
    nc = bass.Bass()

    # columns: [lhsx | rhsy | lhsy | rhsx], each N wide
    aug_d = nc.dram_tensor("aug", [KAUG, 4 * N], bf16, kind="ExternalInput")
    # warm-start duplicate: operands of the first WARMG groups, small enough
    # to land ~6us in while the 768KB main input is still streaming
    warm_d = nc.dram_tensor("warm", [KAUG, WARM_LHS + WARM_RHS], bf16,
                            kind="ExternalInput")
    out_d = nc.dram_tensor("mins", [128, 2 * NBLK], f32, kind="ExternalOutput")

    NG = 2 * (NBLK // GROUP)  # total reduce groups (both sides)

    with (
        nc.sbuf_tensor("aug_sb", [KAUG, 4 * N], bf16) as aug,
        nc.sbuf_tensor("warm_sb", [KAUG, WARM_LHS + WARM_RHS], bf16) as warm,
        nc.sbuf_tensor("mins_sb", [128, 2 * NBLK], f32) as mins,
        nc.sbuf_tensor("stg_sb", [128, 2 * GROUP * W], f32) as stg,
        nc.sbuf_tensor("sc_sb", [128, GROUP * W], f32) as sc,
        nc.psum_tensor("pt_ps", [128, 2 * GROUP * WPAD], f32) as pt,
        nc.semaphore("dma_sem") as dma_sem,
        nc.semaphore("pe_sem") as pe_sem,
        nc.semaphore("dve_sem") as dve_sem,
        nc.semaphore("act_sem") as act_sem,
        nc.semaphore("ckx") as ckx,
        nc.semaphore("cky") as cky,
        nc.semaphore("ckw") as ckw,
        nc.Block() as block,
    ):
        sb = {
            name: aug[:, k * N : (k + 1) * N]
            for k, name in enumerate(("lhsx", "rhsy", "lhsy", "rhsx"))
        }
        sides = ((sb["lhsx"], sb["rhsy"]), (sb["lhsy"], sb["rhsx"]))

        def group_ap(gi, w):
            # [128, GROUP, w] bank-strided view of the (gi % 2) PSUM region
            base = (gi % 2) * GROUP * WPAD
            full = pt[:, base : base + GROUP * WPAD].rearrange(
                "p (g w) -> p g w", w=WPAD
            )
            return full[:, :, 0:w]

        def stg_ap(gi, w):
            # [128, GROUP, w] view of the (gi % 2) bf16 staging buffer
            base = (gi % 2) * GROUP * W
            full = stg[:, base : base + GROUP * W].rearrange(
                "p (g w) -> p g w", w=W
            )
            return full[:, :, 0:w]

        def sc_ap(half, w):
            # ping-pong scratch [128, GROUP, w] (half 0: cols 0:W/2 of each
            # group slot; half 1: cols W/2:W) — DVE-serial use, no sems
            full = sc[:].rearrange("p (g w) -> p g w", w=W)
            return full[:, :, half * (W // 2) : half * (W // 2) + w]

        staged = [gi for gi in range(NG) if gi not in DIRECT]
        cpy_idx = {gi: i for i, gi in enumerate(staged)}

        @block.sync
        def _(sync):
            # warm-start chunk first, then x-side (lhsx|rhsy), then y-side
            sync.dma_start(warm[:], warm_d[:]).then_inc(ckw, 16)
            sync.dma_start(aug[:, : 2 * N], aug_d[:, : 2 * N]).then_inc(ckx, 16)
            sync.dma_start(aug[:, 2 * N :], aug_d[:, 2 * N :]).then_inc(cky, 16)
            # first half of the output overlaps side-1 compute
            sync.wait_ge(dve_sem, NG // 2)
            sync.dma_start(out_d[:, :NBLK], mins[:, :NBLK]).then_inc(dma_sem, 16)
            sync.wait_ge(dve_sem, NG)
            sync.dma_start(out_d[:, NBLK:], mins[:, NBLK:]).then_inc(dma_sem, 16)
            sync.wait_ge(dma_sem, 32)

        @block.tensor
        def _(tensor):
            tensor.wait_ge(ckw, 16)
            for gi in range(NG):
                side, g = divmod(gi, NBLK // GROUP)
                if side == 0 and g < WARMG:
                    lhs = warm[:, :WARM_LHS]
                    rhs = warm[:, WARM_LHS:]
                else:
                    lhs, rhs = sides[side]
                if side == 0 and g == WARMG:
                    tensor.wait_ge(ckx, 16)
                if side == 1 and g == 0:
                    tensor.wait_ge(cky, 16)
                if gi >= 2:
                    # WAR: our PSUM region must have been drained by the
                    # consumer of the group two back
                    prev = gi - 2
                    if prev in cpy_idx:
                        tensor.wait_ge(act_sem, cpy_idx[prev] + 1)
                    else:
                        tensor.wait_ge(dve_sem, prev + 1)
                pg = group_ap(gi, W)
                for k in range(GROUP):
                    i = g * GROUP + k
                    lo = _window_lo(i)
                    mm = tensor.matmul(
                        pg[:, k, :],
                        lhs[:, 128 * i : 128 * (i + 1)],
                        rhs[:, lo : lo + W],
                        start=True,
                        stop=True,
                    )
                    if k == GROUP - 1:
                        # MMs complete in pc order; one inc on the last is sound
                        mm.then_inc(pe_sem, 1)

        @block.scalar
        def _(scalar):
            # dummy copy: absorbs the one-time ACT table load (~2.7us)
            # while the input DMA is still in flight
            scalar.copy(sc[0:1, 0:8], sc[0:1, 8:16])
            for gi in staged:
                scalar.wait_ge(pe_sem, gi + 1)
                if gi >= 2:
                    # staging buffer reuse: group gi-2's reduce must be done
                    scalar.wait_ge(dve_sem, gi - 1)
                scalar.copy(stg_ap(gi, W), group_ap(gi, W)).then_inc(act_sem, 1)

        @block.vector
        def _(vector):
            for gi in range(NG):
                out_ap = mins[:, gi * GROUP : (gi + 1) * GROUP]
                if gi in cpy_idx:
                    vector.wait_ge(act_sem, cpy_idx[gi] + 1)
                    if USE_TREE:
                        # bf16 2x-mode TT min tree, ping-pong scratch halves
                        # (DVE-serial: no sync needed between levels)
                        src, w, half = stg_ap(gi, W), W // 2, 0
                        while w >= 11:
                            dst = sc_ap(half, w)
                            vector.tensor_tensor(
                                dst, src[:, :, 0:w], src[:, :, w : 2 * w],
                                mybir.AluOpType.min,
                            )
                            src, w, half = dst, w // 2, 1 - half
                        vector.tensor_reduce(
                            out_ap, src, axis=mybir.AxisListType.X,
                            op=mybir.AluOpType.min,
                        ).then_inc(dve_sem, 1)
                    else:
                        vector.tensor_reduce(
                            out_ap, stg_ap(gi, W), axis=mybir.AxisListType.X,
                            op=mybir.AluOpType.min,
                        ).then_inc(dve_sem, 1)
                else:
                    vector.wait_ge(pe_sem, gi + 1)
                    vector.tensor_reduce(
                        out_ap, group_ap(gi, W), axis=mybir.AxisListType.X,
                        op=mybir.AluOpType.min,
                    ).then_inc(dve_sem, 1)

    _NC_CACHE["nc"] = nc
    return nc


def _split3(a):
    """Three-level bf16 decomposition: a ~ ah + al + al2 (residual ~2^-27|a|)."""
    import ml_dtypes

    bf = ml_dtypes.bfloat16
    f32 = np.float32
    ah = a.astype(bf).astype(f32)
    r = (a - ah).astype(f32)
    al = r.astype(bf).astype(f32)
    al2 = (r - al).astype(bf).astype(f32)
    return ah, al, al2


def _aug_pair(q, c):
    """bf16-split augmented operands: lhs[:,i] . rhs[:,j] = ||q_i - c_j||^2 / 2.

    All bf16 products are exact in fp32, so accumulating the 6 dominant
    cross terms per coordinate plus triple-split norm rows reproduces the
    fp32 distance to ~1e-7 at bf16 matmul speed (K=24 <= 32 rows is the
    same PE cost as K=5).
    """
    f32 = np.float32
    lhs_rows, rhs_rows = [], []
    for d in range(D):
        ah, al, al2 = _split3(q[:, d])
        bh, bl, bl2 = _split3(-c[:, d])
        lhs_rows += [ah, ah, al, al, ah, al2]
        rhs_rows += [bh, bl, bh, bl, bl2, bh]
    qd = 0.5 * (q * q).sum(1, dtype=np.float64)
    cd = 0.5 * (c * c).sum(1, dtype=np.float64)
    ones = np.ones(N, f32)
    qh, ql, ql2 = _split3(qd.astype(f32))
    ch, cl, cl2 = _split3(cd.astype(f32))
    lhs_rows += [qh, ql, ql2, ones, ones, ones]
    rhs_rows += [ones, ones, ones, ch, cl, cl2]
    import ml_dtypes

    return (
        np.stack(lhs_rows).astype(ml_dtypes.bfloat16),
        np.stack(rhs_rows).astype(ml_dtypes.bfloat16),
    )


def _prep_batch(x, y):
    """Sort by coord 0 and build the augmented matmul operands (host side)."""
    xs = x[np.argsort(x[:, 0], kind="stable")]
    ys = y[np.argsort(y[:, 0], kind="stable")]

    lhsx, rhsy = _aug_pair(xs, ys)
    lhsy, rhsx = _aug_pair(ys, xs)
    aug = np.concatenate([lhsx, rhsy, lhsy, rhsx], axis=1)
    warm = np.concatenate([lhsx[:, :WARM_LHS], rhsy[:, :WARM_RHS]], axis=1)
    return xs, ys, {
        "aug": np.ascontiguousarray(aug),
        "warm": np.ascontiguousarray(warm),
    }


def _fix_side(mins, qs, cs):
    """Posterior exactness check + exact host fixup for unproven rows.

    mins: device banded row minima (full P scale) for sorted queries qs
    against sorted candidates cs. Returns exact per-row minima.
    """
    i = np.arange(N) // 128
    lo = np.clip(128 * i + 64 - W // 2, 0, N - W)
    hi = lo + W
    lb = np.full(N, np.inf)
    has_l = lo > 0
    lb[has_l] = np.maximum(0.0, qs[has_l, 0] - cs[lo[has_l] - 1, 0]) ** 2
    has_r = hi < N
    lb[has_r] = np.minimum(
        lb[has_r], np.maximum(0.0, cs[np.minimum(hi[has_r], N - 1), 0] - qs[has_r, 0]) ** 2
    )
    unproven = mins > lb - 1e-5
    if unproven.any():
        rows = np.where(unproven)[0]
        d = qs[rows, None, :].astype(np.float64) - cs[None, :, :].astype(np.float64)
        exact = (d * d).sum(-1).min(1)
        out = mins.copy()
        out[rows] = np.minimum(mins[rows], exact.astype(np.float32))
        return out
    return mins


def _postprocess(results, meta):
    """Combine per-core device outputs into the final scalar."""
    total = 0.0
    for b in range(B):
        xs, ys = meta[b]
        m = results[b]["mins"]  # [128, 2*NBLK]; [p, s*NBLK+i] = min for rank 128*i+p
        mx = 2.0 * np.ascontiguousarray(m[:, :NBLK].T).reshape(N)  # x queries vs y
        my = 2.0 * np.ascontiguousarray(m[:, NBLK:].T).reshape(N)  # y queries vs x
        mx = _fix_side(mx, xs, ys)
        my = _fix_side(my, ys, xs)
        total += mx.mean(dtype=np.float64) + my.mean(dtype=np.float64)
    return np.array(total / B, dtype=np.float32)


def _run(inputs, trace=False):
    p1 = np.ascontiguousarray(np.asarray(inputs["p1"], dtype=np.float32))
    p2 = np.ascontiguousarray(np.asarray(inputs["p2"], dtype=np.float32))
    assert p1.shape == (B, N, D) and p2.shape == (B, N, D)

    in_maps = []
    meta = []
    for b in range(B):
        xs, ys, im = _prep_batch(p1[b], p2[b])
        in_maps.append(im)
        meta.append((xs, ys))

    from concourse.bass_utils import run_bass_kernel_spmd

    nc = _build_nc()
    kw = {}
    if trace:
        kw = dict(trace=True, trace_cores=list(range(N_CORES)))
    res = run_bass_kernel_spmd(nc, in_maps, list(range(N_CORES)), **kw)
    return _postprocess(res.results, meta), res


def kernel(**inputs):
    out, _ = _run(inputs, trace=False)
    return out


def kernel_traced(**inputs):
    """Same as kernel() but also returns BassKernelResults with NTFF timing."""
    return _run(inputs, trace=True)


# revision 49
# speedup vs baseline: 1.1088x; 1.0009x over previous
"""Chamfer distance kernel for Trainium2 (8 NeuronCores, Bass/Tile).

Problem: p1, p2 are [B=8, N=4096, D=3] fp32 point clouds. Output is the
scalar  mean_j(min_i P[b,i,j]) + mean_i(min_j P[b,i,j])  where
P[b,i,j] = ||p1[b,i] - p2[b,j]||^2.

Strategy
--------
Data-parallel over B: core b handles batch b.

Nearest-neighbor structure: on the host each batch's points are sorted by
coordinate 0. Nearest neighbors are then close in *rank*, so instead of the
full [N, N] distance matrix each 128-point block only scans a W-wide window
of rank-adjacent candidates (a banded distance matrix). Both directions
(min over rows / min over cols) are computed as separate banded passes with
the roles of the two point sets swapped, so on-device both reductions are
free-axis `tensor_reduce(min)` ops.

Each distance block is a single matmul via the augmentation
  lhsT rows ~ [x0, x1, x2, ||x||^2/2, 1]
  rhs  rows ~ [-y0, -y1, -y2, 1, ||y||^2/2]
giving P/2 per element; row mins are doubled on the host. The fp32 rows are
triple-split into bf16 components (K=24, see _aug_pair) because the PE runs
fp32 matmuls at 1/4 rate while K<=32 bf16 costs the same as K=5 — this keeps
fp32-level accuracy (~1e-7) at full bf16 speed.

Exactness: banding alone can miss isolated points. For each row the host
runs an O(1) posterior bound check — every candidate outside the window has
dist^2 >= (coord0 gap to the window edge)^2, so a row whose banded min is
below that gap is *provably* exact. The few unproven rows (~1.5% at W=352 on
randn data) are recomputed exactly on the host with a full scan.
"""

import sys

import numpy as np

if "/opt/trn_rl_repo" not in sys.path:
    sys.path.insert(0, "/opt/trn_rl_repo")

B = 8
N = 4096
D = 3
W = 320          # band width (candidates per 128-row block)
WPAD = 512       # PSUM bank stride per block (fp32 elems; 2KB bank)
NBLK = N // 128  # 32 row blocks per side
GROUP = 4        # blocks reduced per tensor_reduce (4 PSUM banks)
N_CORES = 8
KAUG = 24        # bf16-split augmented contraction dim (see _aug_pair)
WARMG = 4        # leading x-side groups served from the warm-start chunk
WARM_LHS = 128 * GROUP * WARMG                      # lhsx cols duplicated
WARM_RHS = 128 * (GROUP * WARMG - 1) + 64 + W // 2  # rhsy cols duplicated
# reduce strategy: groups in DIRECT tensor_reduce straight from PSUM on DVE;
# the rest are drained PSUM->bf16 SBUF by the otherwise-idle ScalarE, then
# min-reduced on DVE (tree at 2x TT mode if USE_TREE, else plain reduce)
DIRECT = frozenset(range(16))  # staging via ScalarE measured no faster:
USE_TREE = False               # the strided PSUM copy pays the 172cyc
                               # read-write bubble per bank, matching the
                               # direct reduce; keep the simple path

_NC_CACHE = {}


def _window_lo(i):
    # y-rank window start for x-rank block i (static, data independent)
    return min(max(128 * i + 64 - W // 2, 0), N - W)


def _build_nc():
    """Build the (per-core SPMD) Bass program. Cached per process.

    Raw Bass (no Tile): the pipeline is PE (banded matmul groups) -> DVE
    (grouped free-axis min reduce) -> SYNC (DMA out), double-buffered over
    two 4-bank PSUM regions with explicit semaphores. Tile's scheduler
    piggybacks >1 sem wait on compute instructions here, which the walrus
    codegen rejects; standalone wait_ge has no such limit.
    """
    if "nc" in _NC_CACHE:
        return _NC_CACHE["nc"]

    import concourse.bass as bass
    import concourse.mybir as mybir

    f32 = mybir.dt.float32
    bf16 = mybir.dt.bfloat16
    nc = bass.Bass()

    # columns: [lhsx | rhsy | lhsy | rhsx], each N wide
    aug_d = nc.dram_tensor("aug", [KAUG, 4 * N], bf16, kind="ExternalInput")
    # warm-start duplicate: operands of the first WARMG groups, small enough
    # to land ~6us in while the 768KB main input is still streaming
    warm_d = nc.dram_tensor("warm", [KAUG, WARM_LHS + WARM_RHS], bf16,
                            kind="ExternalInput")
    out_d = nc.dram_tensor("mins", [128, 2 * NBLK], f32, kind="ExternalOutput")

    NG = 2 * (NBLK // GROUP)  # total reduce groups (both sides)

    with (
        nc.sbuf_tensor("aug_sb", [KAUG, 4 * N], bf16) as aug,
        nc.sbuf_tensor("warm_sb", [KAUG, WARM_LHS + WARM_RHS], bf16) as warm,
        nc.sbuf_tensor("mins_sb", [128, 2 * NBLK], f32) as mins,
        nc.sbuf_tensor("stg_sb", [128, 2 * GROUP * W], f32) as stg,
        nc.sbuf_tensor("sc_sb", [128, GROUP * W], f32) as sc,
        nc.psum_tensor("pt_ps", [128, 2 * GROUP * WPAD], f32) as pt,
        nc.semaphore("dma_sem") as dma_sem,
        nc.semaphore("pe_sem") as pe_sem,
        nc.semaphore("dve_sem") as dve_sem,
        nc.semaphore("act_sem") as act_sem,
        nc.semaphore("ckx") as ckx,
        nc.semaphore("cky") as cky,
        nc.semaphore("ckw") as ckw,
        nc.Block() as block,
    ):
        sb = {
            name: aug[:, k * N : (k + 1) * N]
            for k, name in enumerate(("lhsx", "rhsy", "lhsy", "rhsx"))
        }
        sides = ((sb["lhsx"], sb["rhsy"]), (sb["lhsy"], sb["rhsx"]))

        def group_ap(gi, w):
            # [128, GROUP, w] bank-strided view of the (gi % 2) PSUM region
            base = (gi % 2) * GROUP * WPAD
            full = pt[:, base : base + GROUP * WPAD].rearrange(
                "p (g w) -> p g w", w=WPAD
            )
            return full[:, :, 0:w]

        def stg_ap(gi, w):
            # [128, GROUP, w] view of the (gi % 2) bf16 staging buffer
            base = (gi % 2) * GROUP * W
            full = stg[:, base : base + GROUP * W].rearrange(
                "p (g w) -> p g w", w=W
            )
            return full[:, :, 0:w]

        def sc_ap(half, w):
            # ping-pong scratch [128, GROUP, w] (half 0: cols 0:W/2 of each
            # group slot; half 1: cols W/2:W) — DVE-serial use, no sems
            full = sc[:].rearrange("p (g w) -> p g w", w=W)
            return full[:, :, half * (W // 2) : half * (W // 2) + w]

        staged = [gi for gi in range(NG) if gi not in DIRECT]
        cpy_idx = {gi: i for i, gi in enumerate(staged)}

        @block.sync
        def _(sync):
            # warm-start chunk first, then x-side (lhsx|rhsy), then y-side
            sync.dma_start(warm[:], warm_d[:]).then_inc(ckw, 16)
            sync.dma_start(aug[:, : 2 * N], aug_d[:, : 2 * N]).then_inc(ckx, 16)
            sync.dma_start(aug[:, 2 * N :], aug_d[:, 2 * N :]).then_inc(cky, 16)
            # stream the output behind the reduces: only the last 16 block
            # minima (8KB) wait for the final reduce group
            sync.wait_ge(dve_sem, NG // 2)
            sync.dma_start(out_d[:, :NBLK], mins[:, :NBLK]).then_inc(dma_sem, 16)
            sync.wait_ge(dve_sem, NG - 1)
            sync.dma_start(
                out_d[:, NBLK : NBLK + 28], mins[:, NBLK : NBLK + 28]
            ).then_inc(dma_sem, 16)
            sync.wait_ge(dve_sem, NG)
            sync.dma_start(
                out_d[:, NBLK + 28 :], mins[:, NBLK + 28 :]
            ).then_inc(dma_sem, 16)
            sync.wait_ge(dma_sem, 48)

        @block.tensor
        def _(tensor):
            tensor.wait_ge(ckw, 16)
            for gi in range(NG):
                side, g = divmod(gi, NBLK // GROUP)
                if side == 0 and g < WARMG:
                    lhs = warm[:, :WARM_LHS]
                    rhs = warm[:, WARM_LHS:]
                else:
                    lhs, rhs = sides[side]
                if side == 0 and g == WARMG:
                    tensor.wait_ge(ckx, 16)
                if side == 1 and g == 0:
                    tensor.wait_ge(cky, 16)
                if gi >= 2:
                    # WAR: our PSUM region must have been drained by the
                    # consumer of the group two back
                    prev = gi - 2
                    if prev in cpy_idx:
                        tensor.wait_ge(act_sem, cpy_idx[prev] + 1)
                    else:
                        tensor.wait_ge(dve_sem, prev + 1)
                pg = group_ap(gi, W)
                for k in range(GROUP):
                    i = g * GROUP + k
                    lo = _window_lo(i)
                    mm = tensor.matmul(
                        pg[:, k, :],
                        lhs[:, 128 * i : 128 * (i + 1)],
                        rhs[:, lo : lo + W],
                        start=True,
                        stop=True,
                    )
                    if k == GROUP - 1:
                        # MMs complete in pc order; one inc on the last is sound
                        mm.then_inc(pe_sem, 1)

        @block.scalar
        def _(scalar):
            # dummy copy: absorbs the one-time ACT table load (~2.7us)
            # while the input DMA is still in flight
            scalar.copy(sc[0:1, 0:8], sc[0:1, 8:16])
            for gi in staged:
                scalar.wait_ge(pe_sem, gi + 1)
                if gi >= 2:
                    # staging buffer reuse: group gi-2's reduce must be done
                    scalar.wait_ge(dve_sem, gi - 1)
                scalar.copy(stg_ap(gi, W), group_ap(gi, W)).then_inc(act_sem, 1)

        @block.vector
        def _(vector):
            for gi in range(NG):
                out_ap = mins[:, gi * GROUP : (gi + 1) * GROUP]
                if gi in cpy_idx:
                    vector.wait_ge(act_sem, cpy_idx[gi] + 1)
                    if USE_TREE:
                        # bf16 2x-mode TT min tree, ping-pong scratch halves
                        # (DVE-serial: no sync needed between levels)
                        src, w, half = stg_ap(gi, W), W // 2, 0
                        while w >= 11:
                            dst = sc_ap(half, w)
                            vector.tensor_tensor(
                                dst, src[:, :, 0:w], src[:, :, w : 2 * w],
                                mybir.AluOpType.min,
                            )
                            src, w, half = dst, w // 2, 1 - half
                        vector.tensor_reduce(
                            out_ap, src, axis=mybir.AxisListType.X,
                            op=mybir.AluOpType.min,
                        ).then_inc(dve_sem, 1)
                    else:
                        vector.tensor_reduce(
                            out_ap, stg_ap(gi, W), axis=mybir.AxisListType.X,
                            op=mybir.AluOpType.min,
                        ).then_inc(dve_sem, 1)
                else:
                    vector.wait_ge(pe_sem, gi + 1)
                    vector.tensor_reduce(
                        out_ap, group_ap(gi, W), axis=mybir.AxisListType.X,
                        op=mybir.AluOpType.min,
                    ).then_inc(dve_sem, 1)

    _NC_CACHE["nc"] = nc
    return nc


def _split3(a):
    """Three-level bf16 decomposition: a ~ ah + al + al2 (residual ~2^-27|a|)."""
    import ml_dtypes

    bf = ml_dtypes.bfloat16
    f32 = np.float32
    ah = a.astype(bf).astype(f32)
    r = (a - ah).astype(f32)
    al = r.astype(bf).astype(f32)
    al2 = (r - al).astype(bf).astype(f32)
    return ah, al, al2


def _aug_pair(q, c):
    """bf16-split augmented operands: lhs[:,i] . rhs[:,j] = ||q_i - c_j||^2 / 2.

    All bf16 products are exact in fp32, so accumulating the 6 dominant
    cross terms per coordinate plus triple-split norm rows reproduces the
    fp32 distance to ~1e-7 at bf16 matmul speed (K=24 <= 32 rows is the
    same PE cost as K=5).
    """
    f32 = np.float32
    lhs_rows, rhs_rows = [], []
    for d in range(D):
        ah, al, al2 = _split3(q[:, d])
        bh, bl, bl2 = _split3(-c[:, d])
        lhs_rows += [ah, ah, al, al, ah, al2]
        rhs_rows += [bh, bl, bh, bl, bl2, bh]
    qd = 0.5 * (q * q).sum(1, dtype=np.float64)
    cd = 0.5 * (c * c).sum(1, dtype=np.float64)
    ones = np.ones(N, f32)
    qh, ql, ql2 = _split3(qd.astype(f32))
    ch, cl, cl2 = _split3(cd.astype(f32))
    lhs_rows += [qh, ql, ql2, ones, ones, ones]
    rhs_rows += [ones, ones, ones, ch, cl, cl2]
    import ml_dtypes

    return (
        np.stack(lhs_rows).astype(ml_dtypes.bfloat16),
        np.stack(rhs_rows).astype(ml_dtypes.bfloat16),
    )


def _prep_batch(x, y):
    """Sort by coord 0 and build the augmented matmul operands (host side)."""
    xs = x[np.argsort(x[:, 0], kind="stable")]
    ys = y[np.argsort(y[:, 0], kind="stable")]

    lhsx, rhsy = _aug_pair(xs, ys)
    lhsy, rhsx = _aug_pair(ys, xs)
    aug = np.concatenate([lhsx, rhsy, lhsy, rhsx], axis=1)
    warm = np.concatenate([lhsx[:, :WARM_LHS], rhsy[:, :WARM_RHS]], axis=1)
    return xs, ys, {
        "aug": np.ascontiguousarray(aug),
        "warm": np.ascontiguousarray(warm),
    }


def _fix_side(mins, qs, cs):
    """Posterior exactness check + exact host fixup for unproven rows.

    mins: device banded row minima (full P scale) for sorted queries qs
    against sorted candidates cs. Returns exact per-row minima.
    """
    i = np.arange(N) // 128
    lo = np.clip(128 * i + 64 - W // 2, 0, N - W)
    hi = lo + W
    lb = np.full(N, np.inf)
    has_l = lo > 0
    lb[has_l] = np.maximum(0.0, qs[has_l, 0] - cs[lo[has_l] - 1, 0]) ** 2
    has_r = hi < N
    lb[has_r] = np.minimum(
        lb[has_r], np.maximum(0.0, cs[np.minimum(hi[has_r], N - 1), 0] - qs[has_r, 0]) ** 2
    )
    unproven = mins > lb - 1e-5
    if unproven.any():
        rows = np.where(unproven)[0]
        d = qs[rows, None, :].astype(np.float64) - cs[None, :, :].astype(np.float64)
        exact = (d * d).sum(-1).min(1)
        out = mins.copy()
        out[rows] = np.minimum(mins[rows], exact.astype(np.float32))
        return out
    return mins


def _postprocess(results, meta):
    """Combine per-core device outputs into the final scalar."""
    total = 0.0
    for b in range(B):
        xs, ys = meta[b]
        m = results[b]["mins"]  # [128, 2*NBLK]; [p, s*NBLK+i] = min for rank 128*i+p
        mx = 2.0 * np.ascontiguousarray(m[:, :NBLK].T).reshape(N)  # x queries vs y
        my = 2.0 * np.ascontiguousarray(m[:, NBLK:].T).reshape(N)  # y queries vs x
        mx = _fix_side(mx, xs, ys)
        my = _fix_side(my, ys, xs)
        total += mx.mean(dtype=np.float64) + my.mean(dtype=np.float64)
    return np.array(total / B, dtype=np.float32)


def _run(inputs, trace=False):
    p1 = np.ascontiguousarray(np.asarray(inputs["p1"], dtype=np.float32))
    p2 = np.ascontiguousarray(np.asarray(inputs["p2"], dtype=np.float32))
    assert p1.shape == (B, N, D) and p2.shape == (B, N, D)

    in_maps = []
    meta = []
    for b in range(B):
        xs, ys, im = _prep_batch(p1[b], p2[b])
        in_maps.append(im)
        meta.append((xs, ys))

    from concourse.bass_utils import run_bass_kernel_spmd

    nc = _build_nc()
    kw = {}
    if trace:
        kw = dict(trace=True, trace_cores=list(range(N_CORES)))
    res = run_bass_kernel_spmd(nc, in_maps, list(range(N_CORES)), **kw)
    return _postprocess(res.results, meta), res


def kernel(**inputs):
    out, _ = _run(inputs, trace=False)
    return out


def kernel_traced(**inputs):
    """Same as kernel() but also returns BassKernelResults with NTFF timing."""
    return _run(inputs, trace=True)


# revision 54
# speedup vs baseline: 1.2469x; 1.1246x over previous
"""Chamfer distance kernel for Trainium2 (8 NeuronCores, Bass/Tile).

Problem: p1, p2 are [B=8, N=4096, D=3] fp32 point clouds. Output is the
scalar  mean_j(min_i P[b,i,j]) + mean_i(min_j P[b,i,j])  where
P[b,i,j] = ||p1[b,i] - p2[b,j]||^2.

Strategy
--------
Data-parallel over B: core b handles batch b.

Nearest-neighbor structure: on the host each batch's points are sorted by
coordinate 0. Nearest neighbors are then close in *rank*, so instead of the
full [N, N] distance matrix each 128-point block only scans a W-wide window
of rank-adjacent candidates (a banded distance matrix). Both directions
(min over rows / min over cols) are computed as separate banded passes with
the roles of the two point sets swapped, so on-device both reductions are
free-axis `tensor_reduce(min)` ops.

Each distance block is a single matmul via the augmentation
  lhsT rows ~ [x0, x1, x2, ||x||^2/2, 1]
  rhs  rows ~ [-y0, -y1, -y2, 1, ||y||^2/2]
giving P/2 per element; row mins are doubled on the host. The fp32 rows are
triple-split into bf16 components (K=24, see _aug_pair) because the PE runs
fp32 matmuls at 1/4 rate while K<=32 bf16 costs the same as K=5 — this keeps
fp32-level accuracy (~1e-7) at full bf16 speed.

Exactness: banding alone can miss isolated points. For each row the host
runs an O(1) posterior bound check — every candidate outside the window has
dist^2 >= (coord0 gap to the window edge)^2, so a row whose banded min is
below that gap is *provably* exact. The few unproven rows (~1.5% at W=352 on
randn data) are recomputed exactly on the host with a full scan.
"""

import sys

import numpy as np

if "/opt/trn_rl_repo" not in sys.path:
    sys.path.insert(0, "/opt/trn_rl_repo")

B = 8
N = 4096
D = 3
W = 256          # band width (candidates per 128-row block)
WPAD = 256       # PSUM stride per block (2 blocks pack one 2KB bank)
NBLK = N // 128  # 32 row blocks per side
GROUP = 8        # blocks reduced per tensor_reduce (4 PSUM banks)
N_CORES = 8
KAUG = 24        # bf16-split augmented contraction dim (see _aug_pair)
WARMG = 1        # leading x-side groups served from the warm-start chunk
WARM_LHS = 128 * GROUP * WARMG                      # lhsx cols duplicated
WARM_RHS = 128 * (GROUP * WARMG - 1) + 64 + W // 2  # rhsy cols duplicated
# reduce strategy: groups in DIRECT tensor_reduce straight from PSUM on DVE;
# the rest are drained PSUM->bf16 SBUF by the otherwise-idle ScalarE, then
# min-reduced on DVE (tree at 2x TT mode if USE_TREE, else plain reduce)
DIRECT = frozenset(range(16))  # staging via ScalarE measured no faster:
USE_TREE = False               # the strided PSUM copy pays the 172cyc
                               # read-write bubble per bank, matching the
                               # direct reduce; keep the simple path

_NC_CACHE = {}


def _window_lo(i):
    # y-rank window start for x-rank block i (static, data independent)
    return min(max(128 * i + 64 - W // 2, 0), N - W)


def _build_nc():
    """Build the (per-core SPMD) Bass program. Cached per process.

    Raw Bass (no Tile): the pipeline is PE (banded matmul groups) -> DVE
    (grouped free-axis min reduce) -> SYNC (DMA out), double-buffered over
    two 4-bank PSUM regions with explicit semaphores. Tile's scheduler
    piggybacks >1 sem wait on compute instructions here, which the walrus
    codegen rejects; standalone wait_ge has no such limit.
    """
    if "nc" in _NC_CACHE:
        return _NC_CACHE["nc"]

    import concourse.bass as bass
    import concourse.mybir as mybir

    f32 = mybir.dt.float32
    bf16 = mybir.dt.bfloat16
    nc = bass.Bass()

    # columns: [lhsx | rhsy | lhsy | rhsx], each N wide
    aug_d = nc.dram_tensor("aug", [KAUG, 4 * N], bf16, kind="ExternalInput")
    # warm-start duplicate: operands of the first WARMG groups, small enough
    # to land ~6us in while the 768KB main input is still streaming
    warm_d = nc.dram_tensor("warm", [KAUG, WARM_LHS + WARM_RHS], bf16,
                            kind="ExternalInput")
    out_d = nc.dram_tensor("mins", [128, 2 * NBLK], f32, kind="ExternalOutput")

    NG = 2 * (NBLK // GROUP)  # total reduce groups (both sides)

    with (
        nc.sbuf_tensor("aug_sb", [KAUG, 4 * N], bf16) as aug,
        nc.sbuf_tensor("warm_sb", [KAUG, WARM_LHS + WARM_RHS], bf16) as warm,
        nc.sbuf_tensor("mins_sb", [128, 2 * NBLK], f32) as mins,
        nc.sbuf_tensor("stg_sb", [128, 2 * GROUP * W], f32) as stg,
        nc.sbuf_tensor("sc_sb", [128, GROUP * W], f32) as sc,
        nc.psum_tensor("pt_ps", [128, 2 * GROUP * WPAD], f32) as pt,
        nc.semaphore("dma_sem") as dma_sem,
        nc.semaphore("pe_sem") as pe_sem,
        nc.semaphore("dve_sem") as dve_sem,
        nc.semaphore("act_sem") as act_sem,
        nc.semaphore("ckx") as ckx,
        nc.semaphore("cky") as cky,
        nc.semaphore("ckw") as ckw,
        nc.Block() as block,
    ):
        sb = {
            name: aug[:, k * N : (k + 1) * N]
            for k, name in enumerate(("lhsx", "rhsy", "lhsy", "rhsx"))
        }
        sides = ((sb["lhsx"], sb["rhsy"]), (sb["lhsy"], sb["rhsx"]))

        def group_ap(gi, w):
            # [128, GROUP, w] bank-strided view of the (gi % 2) PSUM region
            base = (gi % 2) * GROUP * WPAD
            full = pt[:, base : base + GROUP * WPAD].rearrange(
                "p (g w) -> p g w", w=WPAD
            )
            return full[:, :, 0:w]

        def stg_ap(gi, w):
            # [128, GROUP, w] view of the (gi % 2) bf16 staging buffer
            base = (gi % 2) * GROUP * W
            full = stg[:, base : base + GROUP * W].rearrange(
                "p (g w) -> p g w", w=W
            )
            return full[:, :, 0:w]

        def sc_ap(half, w):
            # ping-pong scratch [128, GROUP, w] (half 0: cols 0:W/2 of each
            # group slot; half 1: cols W/2:W) — DVE-serial use, no sems
            full = sc[:].rearrange("p (g w) -> p g w", w=W)
            return full[:, :, half * (W // 2) : half * (W // 2) + w]

        staged = [gi for gi in range(NG) if gi not in DIRECT]
        cpy_idx = {gi: i for i, gi in enumerate(staged)}

        @block.sync
        def _(sync):
            # warm-start chunk first, then x-side (lhsx|rhsy), then y-side
            sync.dma_start(warm[:], warm_d[:]).then_inc(ckw, 16)
            sync.dma_start(aug[:, : 2 * N], aug_d[:, : 2 * N]).then_inc(ckx, 16)
            sync.dma_start(aug[:, 2 * N :], aug_d[:, 2 * N :]).then_inc(cky, 16)
            # stream the output behind the reduces: only the last 16 block
            # minima (8KB) wait for the final reduce group
            sync.wait_ge(dve_sem, NG // 2)
            sync.dma_start(out_d[:, :NBLK], mins[:, :NBLK]).then_inc(dma_sem, 16)
            split1 = NBLK + (NG - 1 - NG // 2) * GROUP
            sync.wait_ge(dve_sem, NG - 1)
            sync.dma_start(
                out_d[:, NBLK:split1], mins[:, NBLK:split1]
            ).then_inc(dma_sem, 16)
            sync.wait_ge(dve_sem, NG)
            sync.dma_start(
                out_d[:, split1:], mins[:, split1:]
            ).then_inc(dma_sem, 16)
            sync.wait_ge(dma_sem, 48)

        @block.tensor
        def _(tensor):
            tensor.wait_ge(ckw, 16)
            for gi in range(NG):
                side, g = divmod(gi, NBLK // GROUP)
                if side == 0 and g < WARMG:
                    lhs = warm[:, :WARM_LHS]
                    rhs = warm[:, WARM_LHS:]
                else:
                    lhs, rhs = sides[side]
                if side == 0 and g == WARMG:
                    tensor.wait_ge(ckx, 16)
                if side == 1 and g == 0:
                    tensor.wait_ge(cky, 16)
                if gi >= 2:
                    # WAR: our PSUM region must have been drained by the
                    # consumer of the group two back
                    prev = gi - 2
                    if prev in cpy_idx:
                        tensor.wait_ge(act_sem, cpy_idx[prev] + 1)
                    else:
                        tensor.wait_ge(dve_sem, prev + 1)
                pg = group_ap(gi, W)
                for k in range(GROUP):
                    i = g * GROUP + k
                    lo = _window_lo(i)
                    mm = tensor.matmul(
                        pg[:, k, :],
                        lhs[:, 128 * i : 128 * (i + 1)],
                        rhs[:, lo : lo + W],
                        start=True,
                        stop=True,
                    )
                    if k == GROUP - 1:
                        # MMs complete in pc order; one inc on the last is sound
                        mm.then_inc(pe_sem, 1)

        @block.scalar
        def _(scalar):
            # dummy copy: absorbs the one-time ACT table load (~2.7us)
            # while the input DMA is still in flight
            scalar.copy(sc[0:1, 0:8], sc[0:1, 8:16])
            for gi in staged:
                scalar.wait_ge(pe_sem, gi + 1)
                if gi >= 2:
                    # staging buffer reuse: group gi-2's reduce must be done
                    scalar.wait_ge(dve_sem, gi - 1)
                scalar.copy(stg_ap(gi, W), group_ap(gi, W)).then_inc(act_sem, 1)

        @block.vector
        def _(vector):
            for gi in range(NG):
                out_ap = mins[:, gi * GROUP : (gi + 1) * GROUP]
                if gi in cpy_idx:
                    vector.wait_ge(act_sem, cpy_idx[gi] + 1)
                    if USE_TREE:
                        # bf16 2x-mode TT min tree, ping-pong scratch halves
                        # (DVE-serial: no sync needed between levels)
                        src, w, half = stg_ap(gi, W), W // 2, 0
                        while w >= 11:
                            dst = sc_ap(half, w)
                            vector.tensor_tensor(
                                dst, src[:, :, 0:w], src[:, :, w : 2 * w],
                                mybir.AluOpType.min,
                            )
                            src, w, half = dst, w // 2, 1 - half
                        vector.tensor_reduce(
                            out_ap, src, axis=mybir.AxisListType.X,
                            op=mybir.AluOpType.min,
                        ).then_inc(dve_sem, 1)
                    else:
                        vector.tensor_reduce(
                            out_ap, stg_ap(gi, W), axis=mybir.AxisListType.X,
                            op=mybir.AluOpType.min,
                        ).then_inc(dve_sem, 1)
                else:
                    vector.wait_ge(pe_sem, gi + 1)
                    vector.tensor_reduce(
                        out_ap, group_ap(gi, W), axis=mybir.AxisListType.X,
                        op=mybir.AluOpType.min,
                    ).then_inc(dve_sem, 1)

    _NC_CACHE["nc"] = nc
    return nc


def _split3(a):
    """Three-level bf16 decomposition: a ~ ah + al + al2 (residual ~2^-27|a|)."""
    import ml_dtypes

    bf = ml_dtypes.bfloat16
    f32 = np.float32
    ah = a.astype(bf).astype(f32)
    r = (a - ah).astype(f32)
    al = r.astype(bf).astype(f32)
    al2 = (r - al).astype(bf).astype(f32)
    return ah, al, al2


def _aug_pair(q, c):
    """bf16-split augmented operands: lhs[:,i] . rhs[:,j] = ||q_i - c_j||^2 / 2.

    All bf16 products are exact in fp32, so accumulating the 6 dominant
    cross terms per coordinate plus triple-split norm rows reproduces the
    fp32 distance to ~1e-7 at bf16 matmul speed (K=24 <= 32 rows is the
    same PE cost as K=5).
    """
    f32 = np.float32
    lhs_rows, rhs_rows = [], []
    for d in range(D):
        ah, al, al2 = _split3(q[:, d])
        bh, bl, bl2 = _split3(-c[:, d])
        lhs_rows += [ah, ah, al, al, ah, al2]
        rhs_rows += [bh, bl, bh, bl, bl2, bh]
    qd = 0.5 * (q * q).sum(1, dtype=np.float64)
    cd = 0.5 * (c * c).sum(1, dtype=np.float64)
    ones = np.ones(N, f32)
    qh, ql, ql2 = _split3(qd.astype(f32))
    ch, cl, cl2 = _split3(cd.astype(f32))
    lhs_rows += [qh, ql, ql2, ones, ones, ones]
    rhs_rows += [ones, ones, ones, ch, cl, cl2]
    import ml_dtypes

    return (
        np.stack(lhs_rows).astype(ml_dtypes.bfloat16),
        np.stack(rhs_rows).astype(ml_dtypes.bfloat16),
    )


def _prep_batch(x, y):
    """Sort by coord 0 and build the augmented matmul operands (host side)."""
    xs = x[np.argsort(x[:, 0], kind="stable")]
    ys = y[np.argsort(y[:, 0], kind="stable")]

    lhsx, rhsy = _aug_pair(xs, ys)
    lhsy, rhsx = _aug_pair(ys, xs)
    aug = np.concatenate([lhsx, rhsy, lhsy, rhsx], axis=1)
    warm = np.concatenate([lhsx[:, :WARM_LHS], rhsy[:, :WARM_RHS]], axis=1)
    return xs, ys, {
        "aug": np.ascontiguousarray(aug),
        "warm": np.ascontiguousarray(warm),
    }


def _fix_side(mins, qs, cs):
    """Posterior exactness check + exact host fixup for unproven rows.

    mins: device banded row minima (full P scale) for sorted queries qs
    against sorted candidates cs. Returns exact per-row minima.
    """
    i = np.arange(N) // 128
    lo = np.clip(128 * i + 64 - W // 2, 0, N - W)
    hi = lo + W
    lb = np.full(N, np.inf)
    has_l = lo > 0
    lb[has_l] = np.maximum(0.0, qs[has_l, 0] - cs[lo[has_l] - 1, 0]) ** 2
    has_r = hi < N
    lb[has_r] = np.minimum(
        lb[has_r], np.maximum(0.0, cs[np.minimum(hi[has_r], N - 1), 0] - qs[has_r, 0]) ** 2
    )
    unproven = mins > lb - 1e-5
    if unproven.any():
        # Exact windowed rescan: the true NN of row r has dist^2 <= mins[r],
        # hence coord0 within +-sqrt(mins[r]) — scan just that slice.
        rows = np.where(unproven)[0]
        c64 = cs.astype(np.float64)
        c0 = c64[:, 0]
        out = mins.copy()
        for r in rows:
            rad = float(np.sqrt(max(mins[r], 0.0) + 1e-6))
            jlo = int(np.searchsorted(c0, qs[r, 0] - rad, "left"))
            jhi = int(np.searchsorted(c0, qs[r, 0] + rad, "right"))
            if jhi > jlo:
                d = c64[jlo:jhi] - qs[r].astype(np.float64)
                out[r] = min(out[r], np.float32((d * d).sum(1).min()))
        return out
    return mins


def _postprocess(results, meta):
    """Combine per-core device outputs into the final scalar."""
    total = 0.0
    for b in range(B):
        xs, ys = meta[b]
        m = results[b]["mins"]  # [128, 2*NBLK]; [p, s*NBLK+i] = min for rank 128*i+p
        mx = 2.0 * np.ascontiguousarray(m[:, :NBLK].T).reshape(N)  # x queries vs y
        my = 2.0 * np.ascontiguousarray(m[:, NBLK:].T).reshape(N)  # y queries vs x
        mx = _fix_side(mx, xs, ys)
        my = _fix_side(my, ys, xs)
        total += mx.mean(dtype=np.float64) + my.mean(dtype=np.float64)
    return np.array(total / B, dtype=np.float32)


def _run(inputs, trace=False):
    p1 = np.ascontiguousarray(np.asarray(inputs["p1"], dtype=np.float32))
    p2 = np.ascontiguousarray(np.asarray(inputs["p2"], dtype=np.float32))
    assert p1.shape == (B, N, D) and p2.shape == (B, N, D)

    in_maps = []
    meta = []
    for b in range(B):
        xs, ys, im = _prep_batch(p1[b], p2[b])
        in_maps.append(im)
        meta.append((xs, ys))

    from concourse.bass_utils import run_bass_kernel_spmd

    nc = _build_nc()
    kw = {}
    if trace:
        kw = dict(trace=True, trace_cores=list(range(N_CORES)))
    res = run_bass_kernel_spmd(nc, in_maps, list(range(N_CORES)), **kw)
    return _postprocess(res.results, meta), res


def kernel(**inputs):
    out, _ = _run(inputs, trace=False)
    return out


def kernel_traced(**inputs):
    """Same as kernel() but also returns BassKernelResults with NTFF timing."""
    return _run(inputs, trace=True)


# revision 55
# speedup vs baseline: 1.3146x; 1.0543x over previous
"""Chamfer distance kernel for Trainium2 (8 NeuronCores, Bass/Tile).

Problem: p1, p2 are [B=8, N=4096, D=3] fp32 point clouds. Output is the
scalar  mean_j(min_i P[b,i,j]) + mean_i(min_j P[b,i,j])  where
P[b,i,j] = ||p1[b,i] - p2[b,j]||^2.

Strategy
--------
Data-parallel over B: core b handles batch b.

Nearest-neighbor structure: on the host each batch's points are sorted by
coordinate 0. Nearest neighbors are then close in *rank*, so instead of the
full [N, N] distance matrix each 128-point block only scans a W-wide window
of rank-adjacent candidates (a banded distance matrix). Both directions
(min over rows / min over cols) are computed as separate banded passes with
the roles of the two point sets swapped, so on-device both reductions are
free-axis `tensor_reduce(min)` ops.

Each distance block is a single matmul via the augmentation
  lhsT rows ~ [x0, x1, x2, ||x||^2/2, 1]
  rhs  rows ~ [-y0, -y1, -y2, 1, ||y||^2/2]
giving P/2 per element; row mins are doubled on the host. The fp32 rows are
triple-split into bf16 components (K=24, see _aug_pair) because the PE runs
fp32 matmuls at 1/4 rate while K<=32 bf16 costs the same as K=5 — this keeps
fp32-level accuracy (~1e-7) at full bf16 speed.

Exactness: banding alone can miss isolated points. For each row the host
runs an O(1) posterior bound check — every candidate outside the window has
dist^2 >= (coord0 gap to the window edge)^2, so a row whose banded min is
below that gap is *provably* exact. The few unproven rows (~1.5% at W=352 on
randn data) are recomputed exactly on the host with a full scan.
"""

import sys

import numpy as np

if "/opt/trn_rl_repo" not in sys.path:
    sys.path.insert(0, "/opt/trn_rl_repo")

B = 8
N = 4096
D = 3
W = 224          # band width (candidates per 128-row block)
WPAD = 256       # PSUM stride per block (2 blocks pack one 2KB bank)
NBLK = N // 128  # 32 row blocks per side
GROUP = 8        # blocks reduced per tensor_reduce (4 PSUM banks)
N_CORES = 8
KAUG = 24        # bf16-split augmented contraction dim (see _aug_pair)
WARMG = 1        # leading x-side groups served from the warm-start chunk
WARM_LHS = 128 * GROUP * WARMG                      # lhsx cols duplicated
WARM_RHS = 128 * (GROUP * WARMG - 1) + 64 + W // 2  # rhsy cols duplicated
# reduce strategy: groups in DIRECT tensor_reduce straight from PSUM on DVE;
# the rest are drained PSUM->bf16 SBUF by the otherwise-idle ScalarE, then
# min-reduced on DVE (tree at 2x TT mode if USE_TREE, else plain reduce)
DIRECT = frozenset(range(16))  # staging via ScalarE measured no faster:
USE_TREE = False               # the strided PSUM copy pays the 172cyc
                               # read-write bubble per bank, matching the
                               # direct reduce; keep the simple path

_NC_CACHE = {}


def _window_lo(i):
    # y-rank window start for x-rank block i (static, data independent)
    return min(max(128 * i + 64 - W // 2, 0), N - W)


def _build_nc():
    """Build the (per-core SPMD) Bass program. Cached per process.

    Raw Bass (no Tile): the pipeline is PE (banded matmul groups) -> DVE
    (grouped free-axis min reduce) -> SYNC (DMA out), double-buffered over
    two 4-bank PSUM regions with explicit semaphores. Tile's scheduler
    piggybacks >1 sem wait on compute instructions here, which the walrus
    codegen rejects; standalone wait_ge has no such limit.
    """
    if "nc" in _NC_CACHE:
        return _NC_CACHE["nc"]

    import concourse.bass as bass
    import concourse.mybir as mybir

    f32 = mybir.dt.float32
    bf16 = mybir.dt.bfloat16
    nc = bass.Bass()

    # columns: [lhsx | rhsy | lhsy | rhsx], each N wide
    aug_d = nc.dram_tensor("aug", [KAUG, 4 * N], bf16, kind="ExternalInput")
    # warm-start duplicate: operands of the first WARMG groups, small enough
    # to land ~6us in while the 768KB main input is still streaming
    warm_d = nc.dram_tensor("warm", [KAUG, WARM_LHS + WARM_RHS], bf16,
                            kind="ExternalInput")
    out_d = nc.dram_tensor("mins", [128, 2 * NBLK], f32, kind="ExternalOutput")

    NG = 2 * (NBLK // GROUP)  # total reduce groups (both sides)

    with (
        nc.sbuf_tensor("aug_sb", [KAUG, 4 * N], bf16) as aug,
        nc.sbuf_tensor("warm_sb", [KAUG, WARM_LHS + WARM_RHS], bf16) as warm,
        nc.sbuf_tensor("mins_sb", [128, 2 * NBLK], f32) as mins,
        nc.sbuf_tensor("stg_sb", [128, 2 * GROUP * W], f32) as stg,
        nc.sbuf_tensor("sc_sb", [128, GROUP * W], f32) as sc,
        nc.psum_tensor("pt_ps", [128, 2 * GROUP * WPAD], f32) as pt,
        nc.semaphore("dma_sem") as dma_sem,
        nc.semaphore("pe_sem") as pe_sem,
        nc.semaphore("dve_sem") as dve_sem,
        nc.semaphore("act_sem") as act_sem,
        nc.semaphore("ckx") as ckx,
        nc.semaphore("cky") as cky,
        nc.semaphore("ckw") as ckw,
        nc.Block() as block,
    ):
        sb = {
            name: aug[:, k * N : (k + 1) * N]
            for k, name in enumerate(("lhsx", "rhsy", "lhsy", "rhsx"))
        }
        sides = ((sb["lhsx"], sb["rhsy"]), (sb["lhsy"], sb["rhsx"]))

        def group_ap(gi, w):
            # [128, GROUP, w] bank-strided view of the (gi % 2) PSUM region
            base = (gi % 2) * GROUP * WPAD
            full = pt[:, base : base + GROUP * WPAD].rearrange(
                "p (g w) -> p g w", w=WPAD
            )
            return full[:, :, 0:w]

        def stg_ap(gi, w):
            # [128, GROUP, w] view of the (gi % 2) bf16 staging buffer
            base = (gi % 2) * GROUP * W
            full = stg[:, base : base + GROUP * W].rearrange(
                "p (g w) -> p g w", w=W
            )
            return full[:, :, 0:w]

        def sc_ap(half, w):
            # ping-pong scratch [128, GROUP, w] (half 0: cols 0:W/2 of each
            # group slot; half 1: cols W/2:W) — DVE-serial use, no sems
            full = sc[:].rearrange("p (g w) -> p g w", w=W)
            return full[:, :, half * (W // 2) : half * (W // 2) + w]

        staged = [gi for gi in range(NG) if gi not in DIRECT]
        cpy_idx = {gi: i for i, gi in enumerate(staged)}

        @block.sync
        def _(sync):
            # warm-start chunk first, then x-side (lhsx|rhsy), then y-side
            sync.dma_start(warm[:], warm_d[:]).then_inc(ckw, 16)
            sync.dma_start(aug[:, : 2 * N], aug_d[:, : 2 * N]).then_inc(ckx, 16)
            sync.dma_start(aug[:, 2 * N :], aug_d[:, 2 * N :]).then_inc(cky, 16)
            # stream the output behind the reduces: only the last 16 block
            # minima (8KB) wait for the final reduce group
            sync.wait_ge(dve_sem, NG // 2)
            sync.dma_start(out_d[:, :NBLK], mins[:, :NBLK]).then_inc(dma_sem, 16)
            split1 = NBLK + (NG - 1 - NG // 2) * GROUP
            sync.wait_ge(dve_sem, NG - 1)
            sync.dma_start(
                out_d[:, NBLK:split1], mins[:, NBLK:split1]
            ).then_inc(dma_sem, 16)
            sync.wait_ge(dve_sem, NG)
            sync.dma_start(
                out_d[:, split1:], mins[:, split1:]
            ).then_inc(dma_sem, 16)
            sync.wait_ge(dma_sem, 48)

        @block.tensor
        def _(tensor):
            tensor.wait_ge(ckw, 16)
            for gi in range(NG):
                side, g = divmod(gi, NBLK // GROUP)
                if side == 0 and g < WARMG:
                    lhs = warm[:, :WARM_LHS]
                    rhs = warm[:, WARM_LHS:]
                else:
                    lhs, rhs = sides[side]
                if side == 0 and g == WARMG:
                    tensor.wait_ge(ckx, 16)
                if side == 1 and g == 0:
                    tensor.wait_ge(cky, 16)
                if gi >= 2:
                    # WAR: our PSUM region must have been drained by the
                    # consumer of the group two back
                    prev = gi - 2
                    if prev in cpy_idx:
                        tensor.wait_ge(act_sem, cpy_idx[prev] + 1)
                    else:
                        tensor.wait_ge(dve_sem, prev + 1)
                pg = group_ap(gi, W)
                for k in range(GROUP):
                    i = g * GROUP + k
                    lo = _window_lo(i)
                    mm = tensor.matmul(
                        pg[:, k, :],
                        lhs[:, 128 * i : 128 * (i + 1)],
                        rhs[:, lo : lo + W],
                        start=True,
                        stop=True,
                    )
                    if k == GROUP - 1:
                        # MMs complete in pc order; one inc on the last is sound
                        mm.then_inc(pe_sem, 1)

        @block.scalar
        def _(scalar):
            # dummy copy: absorbs the one-time ACT table load (~2.7us)
            # while the input DMA is still in flight
            scalar.copy(sc[0:1, 0:8], sc[0:1, 8:16])
            for gi in staged:
                scalar.wait_ge(pe_sem, gi + 1)
                if gi >= 2:
                    # staging buffer reuse: group gi-2's reduce must be done
                    scalar.wait_ge(dve_sem, gi - 1)
                scalar.copy(stg_ap(gi, W), group_ap(gi, W)).then_inc(act_sem, 1)

        @block.vector
        def _(vector):
            for gi in range(NG):
                out_ap = mins[:, gi * GROUP : (gi + 1) * GROUP]
                if gi in cpy_idx:
                    vector.wait_ge(act_sem, cpy_idx[gi] + 1)
                    if USE_TREE:
                        # bf16 2x-mode TT min tree, ping-pong scratch halves
                        # (DVE-serial: no sync needed between levels)
                        src, w, half = stg_ap(gi, W), W // 2, 0
                        while w >= 11:
                            dst = sc_ap(half, w)
                            vector.tensor_tensor(
                                dst, src[:, :, 0:w], src[:, :, w : 2 * w],
                                mybir.AluOpType.min,
                            )
                            src, w, half = dst, w // 2, 1 - half
                        vector.tensor_reduce(
                            out_ap, src, axis=mybir.AxisListType.X,
                            op=mybir.AluOpType.min,
                        ).then_inc(dve_sem, 1)
                    else:
                        vector.tensor_reduce(
                            out_ap, stg_ap(gi, W), axis=mybir.AxisListType.X,
                            op=mybir.AluOpType.min,
                        ).then_inc(dve_sem, 1)
                else:
                    vector.wait_ge(pe_sem, gi + 1)
                    vector.tensor_reduce(
                        out_ap, group_ap(gi, W), axis=mybir.AxisListType.X,
                        op=mybir.AluOpType.min,
                    ).then_inc(dve_sem, 1)

    _NC_CACHE["nc"] = nc
    return nc


def _split3(a):
    """Three-level bf16 decomposition: a ~ ah + al + al2 (residual ~2^-27|a|)."""
    import ml_dtypes

    bf = ml_dtypes.bfloat16
    f32 = np.float32
    ah = a.astype(bf).astype(f32)
    r = (a - ah).astype(f32)
    al = r.astype(bf).astype(f32)
    al2 = (r - al).astype(bf).astype(f32)
    return ah, al, al2


def _aug_pair(q, c):
    """bf16-split augmented operands: lhs[:,i] . rhs[:,j] = ||q_i - c_j||^2 / 2.

    All bf16 products are exact in fp32, so accumulating the 6 dominant
    cross terms per coordinate plus triple-split norm rows reproduces the
    fp32 distance to ~1e-7 at bf16 matmul speed (K=24 <= 32 rows is the
    same PE cost as K=5).
    """
    f32 = np.float32
    lhs_rows, rhs_rows = [], []
    for d in range(D):
        ah, al, al2 = _split3(q[:, d])
        bh, bl, bl2 = _split3(-c[:, d])
        lhs_rows += [ah, ah, al, al, ah, al2]
        rhs_rows += [bh, bl, bh, bl, bl2, bh]
    qd = 0.5 * (q * q).sum(1, dtype=np.float64)
    cd = 0.5 * (c * c).sum(1, dtype=np.float64)
    ones = np.ones(N, f32)
    qh, ql, ql2 = _split3(qd.astype(f32))
    ch, cl, cl2 = _split3(cd.astype(f32))
    lhs_rows += [qh, ql, ql2, ones, ones, ones]
    rhs_rows += [ones, ones, ones, ch, cl, cl2]
    import ml_dtypes

    return (
        np.stack(lhs_rows).astype(ml_dtypes.bfloat16),
        np.stack(rhs_rows).astype(ml_dtypes.bfloat16),
    )


def _prep_batch(x, y):
    """Sort by coord 0 and build the augmented matmul operands (host side)."""
    xs = x[np.argsort(x[:, 0], kind="stable")]
    ys = y[np.argsort(y[:, 0], kind="stable")]

    lhsx, rhsy = _aug_pair(xs, ys)
    lhsy, rhsx = _aug_pair(ys, xs)
    aug = np.concatenate([lhsx, rhsy, lhsy, rhsx], axis=1)
    warm = np.concatenate([lhsx[:, :WARM_LHS], rhsy[:, :WARM_RHS]], axis=1)
    return xs, ys, {
        "aug": np.ascontiguousarray(aug),
        "warm": np.ascontiguousarray(warm),
    }


def _fix_side(mins, qs, cs):
    """Posterior exactness check + exact host fixup for unproven rows.

    mins: device banded row minima (full P scale) for sorted queries qs
    against sorted candidates cs. Returns exact per-row minima.
    """
    i = np.arange(N) // 128
    lo = np.clip(128 * i + 64 - W // 2, 0, N - W)
    hi = lo + W
    lb = np.full(N, np.inf)
    has_l = lo > 0
    lb[has_l] = np.maximum(0.0, qs[has_l, 0] - cs[lo[has_l] - 1, 0]) ** 2
    has_r = hi < N
    lb[has_r] = np.minimum(
        lb[has_r], np.maximum(0.0, cs[np.minimum(hi[has_r], N - 1), 0] - qs[has_r, 0]) ** 2
    )
    unproven = mins > lb - 1e-5
    if unproven.any():
        # Exact windowed rescan: the true NN of row r has dist^2 <= mins[r],
        # hence coord0 within +-sqrt(mins[r]) — scan just that slice.
        rows = np.where(unproven)[0]
        c64 = cs.astype(np.float64)
        c0 = c64[:, 0]
        out = mins.copy()
        for r in rows:
            rad = float(np.sqrt(max(mins[r], 0.0) + 1e-6))
            jlo = int(np.searchsorted(c0, qs[r, 0] - rad, "left"))
            jhi = int(np.searchsorted(c0, qs[r, 0] + rad, "right"))
            if jhi > jlo:
                d = c64[jlo:jhi] - qs[r].astype(np.float64)
                out[r] = min(out[r], np.float32((d * d).sum(1).min()))
        return out
    return mins


def _postprocess(results, meta):
    """Combine per-core device outputs into the final scalar."""
    total = 0.0
    for b in range(B):
        xs, ys = meta[b]
        m = results[b]["mins"]  # [128, 2*NBLK]; [p, s*NBLK+i] = min for rank 128*i+p
        mx = 2.0 * np.ascontiguousarray(m[:, :NBLK].T).reshape(N)  # x queries vs y
        my = 2.0 * np.ascontiguousarray(m[:, NBLK:].T).reshape(N)  # y queries vs x
        mx = _fix_side(mx, xs, ys)
        my = _fix_side(my, ys, xs)
        total += mx.mean(dtype=np.float64) + my.mean(dtype=np.float64)
    return np.array(total / B, dtype=np.float32)


def _run(inputs, trace=False):
    p1 = np.ascontiguousarray(np.asarray(inputs["p1"], dtype=np.float32))
    p2 = np.ascontiguousarray(np.asarray(inputs["p2"], dtype=np.float32))
    assert p1.shape == (B, N, D) and p2.shape == (B, N, D)

    in_maps = []
    meta = []
    for b in range(B):
        xs, ys, im = _prep_batch(p1[b], p2[b])
        in_maps.append(im)
        meta.append((xs, ys))

    from concourse.bass_utils import run_bass_kernel_spmd

    nc = _build_nc()
    kw = {}
    if trace:
        kw = dict(trace=True, trace_cores=list(range(N_CORES)))
    res = run_bass_kernel_spmd(nc, in_maps, list(range(N_CORES)), **kw)
    return _postprocess(res.results, meta), res


def kernel(**inputs):
    out, _ = _run(inputs, trace=False)
    return out


def kernel_traced(**inputs):
    """Same as kernel() but also returns BassKernelResults with NTFF timing."""
    return _run(inputs, trace=True)


# revision 56
# speedup vs baseline: 1.3901x; 1.0574x over previous
"""Chamfer distance kernel for Trainium2 (8 NeuronCores, Bass/Tile).

Problem: p1, p2 are [B=8, N=4096, D=3] fp32 point clouds. Output is the
scalar  mean_j(min_i P[b,i,j]) + mean_i(min_j P[b,i,j])  where
P[b,i,j] = ||p1[b,i] - p2[b,j]||^2.

Strategy
--------
Data-parallel over B: core b handles batch b.

Nearest-neighbor structure: on the host each batch's points are sorted by
coordinate 0. Nearest neighbors are then close in *rank*, so instead of the
full [N, N] distance matrix each 128-point block only scans a W-wide window
of rank-adjacent candidates (a banded distance matrix). Both directions
(min over rows / min over cols) are computed as separate banded passes with
the roles of the two point sets swapped, so on-device both reductions are
free-axis `tensor_reduce(min)` ops.

Each distance block is a single matmul via the augmentation
  lhsT rows ~ [x0, x1, x2, ||x||^2/2, 1]
  rhs  rows ~ [-y0, -y1, -y2, 1, ||y||^2/2]
giving P/2 per element; row mins are doubled on the host. The fp32 rows are
triple-split into bf16 components (K=24, see _aug_pair) because the PE runs
fp32 matmuls at 1/4 rate while K<=32 bf16 costs the same as K=5 — this keeps
fp32-level accuracy (~1e-7) at full bf16 speed.

Exactness: banding alone can miss isolated points. For each row the host
runs an O(1) posterior bound check — every candidate outside the window has
dist^2 >= (coord0 gap to the window edge)^2, so a row whose banded min is
below that gap is *provably* exact. The few unproven rows (~1.5% at W=352 on
randn data) are recomputed exactly on the host with a full scan.
"""

import sys

import numpy as np

if "/opt/trn_rl_repo" not in sys.path:
    sys.path.insert(0, "/opt/trn_rl_repo")

B = 8
N = 4096
D = 3
W = 208          # band width (candidates per 128-row block)
WPAD = 256       # PSUM stride per block (2 blocks pack one 2KB bank)
NBLK = N // 128  # 32 row blocks per side
GROUP = 8        # blocks reduced per tensor_reduce (4 PSUM banks)
N_CORES = 8
KAUG = 24        # bf16-split augmented contraction dim (see _aug_pair)
WARMG = 1        # leading x-side groups served from the warm-start chunk
WARM_LHS = 128 * GROUP * WARMG                      # lhsx cols duplicated
WARM_RHS = 128 * (GROUP * WARMG - 1) + 64 + W // 2  # rhsy cols duplicated
# reduce strategy: groups in DIRECT tensor_reduce straight from PSUM on DVE;
# the rest are drained PSUM->bf16 SBUF by the otherwise-idle ScalarE, then
# min-reduced on DVE (tree at 2x TT mode if USE_TREE, else plain reduce)
DIRECT = frozenset(range(16))  # staging via ScalarE measured no faster:
USE_TREE = False               # the strided PSUM copy pays the 172cyc
                               # read-write bubble per bank, matching the
                               # direct reduce; keep the simple path

_NC_CACHE = {}


def _window_lo(i):
    # y-rank window start for x-rank block i (static, data independent)
    return min(max(128 * i + 64 - W // 2, 0), N - W)


def _build_nc():
    """Build the (per-core SPMD) Bass program. Cached per process.

    Raw Bass (no Tile): the pipeline is PE (banded matmul groups) -> DVE
    (grouped free-axis min reduce) -> SYNC (DMA out), double-buffered over
    two 4-bank PSUM regions with explicit semaphores. Tile's scheduler
    piggybacks >1 sem wait on compute instructions here, which the walrus
    codegen rejects; standalone wait_ge has no such limit.
    """
    if "nc" in _NC_CACHE:
        return _NC_CACHE["nc"]

    import concourse.bass as bass
    import concourse.mybir as mybir

    f32 = mybir.dt.float32
    bf16 = mybir.dt.bfloat16
    nc = bass.Bass()

    # columns: [lhsx | rhsy | lhsy | rhsx], each N wide
    aug_d = nc.dram_tensor("aug", [KAUG, 4 * N], bf16, kind="ExternalInput")
    # warm-start duplicate: operands of the first WARMG groups, small enough
    # to land ~6us in while the 768KB main input is still streaming
    warm_d = nc.dram_tensor("warm", [KAUG, WARM_LHS + WARM_RHS], bf16,
                            kind="ExternalInput")
    out_d = nc.dram_tensor("mins", [128, 2 * NBLK], f32, kind="ExternalOutput")

    NG = 2 * (NBLK // GROUP)  # total reduce groups (both sides)

    with (
        nc.sbuf_tensor("aug_sb", [KAUG, 4 * N], bf16) as aug,
        nc.sbuf_tensor("warm_sb", [KAUG, WARM_LHS + WARM_RHS], bf16) as warm,
        nc.sbuf_tensor("mins_sb", [128, 2 * NBLK], f32) as mins,
        nc.sbuf_tensor("stg_sb", [128, 2 * GROUP * W], f32) as stg,
        nc.sbuf_tensor("sc_sb", [128, GROUP * W], f32) as sc,
        nc.psum_tensor("pt_ps", [128, 2 * GROUP * WPAD], f32) as pt,
        nc.semaphore("dma_sem") as dma_sem,
        nc.semaphore("pe_sem") as pe_sem,
        nc.semaphore("dve_sem") as dve_sem,
        nc.semaphore("act_sem") as act_sem,
        nc.semaphore("ckx") as ckx,
        nc.semaphore("cky") as cky,
        nc.semaphore("ckw") as ckw,
        nc.Block() as block,
    ):
        sb = {
            name: aug[:, k * N : (k + 1) * N]
            for k, name in enumerate(("lhsx", "rhsy", "lhsy", "rhsx"))
        }
        sides = ((sb["lhsx"], sb["rhsy"]), (sb["lhsy"], sb["rhsx"]))

        def group_ap(gi, w):
            # [128, GROUP, w] bank-strided view of the (gi % 2) PSUM region
            base = (gi % 2) * GROUP * WPAD
            full = pt[:, base : base + GROUP * WPAD].rearrange(
                "p (g w) -> p g w", w=WPAD
            )
            return full[:, :, 0:w]

        def stg_ap(gi, w):
            # [128, GROUP, w] view of the (gi % 2) bf16 staging buffer
            base = (gi % 2) * GROUP * W
            full = stg[:, base : base + GROUP * W].rearrange(
                "p (g w) -> p g w", w=W
            )
            return full[:, :, 0:w]

        def sc_ap(half, w):
            # ping-pong scratch [128, GROUP, w] (half 0: cols 0:W/2 of each
            # group slot; half 1: cols W/2:W) — DVE-serial use, no sems
            full = sc[:].rearrange("p (g w) -> p g w", w=W)
            return full[:, :, half * (W // 2) : half * (W // 2) + w]

        staged = [gi for gi in range(NG) if gi not in DIRECT]
        cpy_idx = {gi: i for i, gi in enumerate(staged)}

        @block.sync
        def _(sync):
            # warm-start chunk first, then x-side (lhsx|rhsy), then y-side
            sync.dma_start(warm[:], warm_d[:]).then_inc(ckw, 16)
            sync.dma_start(aug[:, : 2 * N], aug_d[:, : 2 * N]).then_inc(ckx, 16)
            sync.dma_start(aug[:, 2 * N :], aug_d[:, 2 * N :]).then_inc(cky, 16)
            # stream the output behind the reduces: only the last 16 block
            # minima (8KB) wait for the final reduce group
            sync.wait_ge(dve_sem, NG // 2)
            sync.dma_start(out_d[:, :NBLK], mins[:, :NBLK]).then_inc(dma_sem, 16)
            split1 = NBLK + (NG - 1 - NG // 2) * GROUP
            sync.wait_ge(dve_sem, NG - 1)
            sync.dma_start(
                out_d[:, NBLK:split1], mins[:, NBLK:split1]
            ).then_inc(dma_sem, 16)
            sync.wait_ge(dve_sem, NG)
            sync.dma_start(
                out_d[:, split1:], mins[:, split1:]
            ).then_inc(dma_sem, 16)
            sync.wait_ge(dma_sem, 48)

        @block.tensor
        def _(tensor):
            tensor.wait_ge(ckw, 16)
            for gi in range(NG):
                side, g = divmod(gi, NBLK // GROUP)
                if side == 0 and g < WARMG:
                    lhs = warm[:, :WARM_LHS]
                    rhs = warm[:, WARM_LHS:]
                else:
                    lhs, rhs = sides[side]
                if side == 0 and g == WARMG:
                    tensor.wait_ge(ckx, 16)
                if side == 1 and g == 0:
                    tensor.wait_ge(cky, 16)
                if gi >= 2:
                    # WAR: our PSUM region must have been drained by the
                    # consumer of the group two back
                    prev = gi - 2
                    if prev in cpy_idx:
                        tensor.wait_ge(act_sem, cpy_idx[prev] + 1)
                    else:
                        tensor.wait_ge(dve_sem, prev + 1)
                pg = group_ap(gi, W)
                for k in range(GROUP):
                    i = g * GROUP + k
                    lo = _window_lo(i)
                    mm = tensor.matmul(
                        pg[:, k, :],
                        lhs[:, 128 * i : 128 * (i + 1)],
                        rhs[:, lo : lo + W],
                        start=True,
                        stop=True,
                    )
                    if k == GROUP - 1:
                        # MMs complete in pc order; one inc on the last is sound
                        mm.then_inc(pe_sem, 1)

        @block.scalar
        def _(scalar):
            # dummy copy: absorbs the one-time ACT table load (~2.7us)
            # while the input DMA is still in flight
            scalar.copy(sc[0:1, 0:8], sc[0:1, 8:16])
            for gi in staged:
                scalar.wait_ge(pe_sem, gi + 1)
                if gi >= 2:
                    # staging buffer reuse: group gi-2's reduce must be done
                    scalar.wait_ge(dve_sem, gi - 1)
                scalar.copy(stg_ap(gi, W), group_ap(gi, W)).then_inc(act_sem, 1)

        @block.vector
        def _(vector):
            for gi in range(NG):
                out_ap = mins[:, gi * GROUP : (gi + 1) * GROUP]
                if gi in cpy_idx:
                    vector.wait_ge(act_sem, cpy_idx[gi] + 1)
                    if USE_TREE:
                        # bf16 2x-mode TT min tree, ping-pong scratch halves
                        # (DVE-serial: no sync needed between levels)
                        src, w, half = stg_ap(gi, W), W // 2, 0
                        while w >= 11:
                            dst = sc_ap(half, w)
                            vector.tensor_tensor(
                                dst, src[:, :, 0:w], src[:, :, w : 2 * w],
                                mybir.AluOpType.min,
                            )
                            src, w, half = dst, w // 2, 1 - half
                        vector.tensor_reduce(
                            out_ap, src, axis=mybir.AxisListType.X,
                            op=mybir.AluOpType.min,
                        ).then_inc(dve_sem, 1)
                    else:
                        vector.tensor_reduce(
                            out_ap, stg_ap(gi, W), axis=mybir.AxisListType.X,
                            op=mybir.AluOpType.min,
                        ).then_inc(dve_sem, 1)
                else:
                    vector.wait_ge(pe_sem, gi + 1)
                    vector.tensor_reduce(
                        out_ap, group_ap(gi, W), axis=mybir.AxisListType.X,
                        op=mybir.AluOpType.min,
                    ).then_inc(dve_sem, 1)

    _NC_CACHE["nc"] = nc
    return nc


def _split3(a):
    """Three-level bf16 decomposition: a ~ ah + al + al2 (residual ~2^-27|a|)."""
    import ml_dtypes

    bf = ml_dtypes.bfloat16
    f32 = np.float32
    ah = a.astype(bf).astype(f32)
    r = (a - ah).astype(f32)
    al = r.astype(bf).astype(f32)
    al2 = (r - al).astype(bf).astype(f32)
    return ah, al, al2


def _aug_pair(q, c):
    """bf16-split augmented operands: lhs[:,i] . rhs[:,j] = ||q_i - c_j||^2 / 2.

    All bf16 products are exact in fp32, so accumulating the 6 dominant
    cross terms per coordinate plus triple-split norm rows reproduces the
    fp32 distance to ~1e-7 at bf16 matmul speed (K=24 <= 32 rows is the
    same PE cost as K=5).
    """
    f32 = np.float32
    lhs_rows, rhs_rows = [], []
    for d in range(D):
        ah, al, al2 = _split3(q[:, d])
        bh, bl, bl2 = _split3(-c[:, d])
        lhs_rows += [ah, ah, al, al, ah, al2]
        rhs_rows += [bh, bl, bh, bl, bl2, bh]
    qd = 0.5 * (q * q).sum(1, dtype=np.float64)
    cd = 0.5 * (c * c).sum(1, dtype=np.float64)
    ones = np.ones(N, f32)
    qh, ql, ql2 = _split3(qd.astype(f32))
    ch, cl, cl2 = _split3(cd.astype(f32))
    lhs_rows += [qh, ql, ql2, ones, ones, ones]
    rhs_rows += [ones, ones, ones, ch, cl, cl2]
    import ml_dtypes

    return (
        np.stack(lhs_rows).astype(ml_dtypes.bfloat16),
        np.stack(rhs_rows).astype(ml_dtypes.bfloat16),
    )


def _prep_batch(x, y):
    """Sort by coord 0 and build the augmented matmul operands (host side)."""
    xs = x[np.argsort(x[:, 0], kind="stable")]
    ys = y[np.argsort(y[:, 0], kind="stable")]

    lhsx, rhsy = _aug_pair(xs, ys)
    lhsy, rhsx = _aug_pair(ys, xs)
    aug = np.concatenate([lhsx, rhsy, lhsy, rhsx], axis=1)
    warm = np.concatenate([lhsx[:, :WARM_LHS], rhsy[:, :WARM_RHS]], axis=1)
    return xs, ys, {
        "aug": np.ascontiguousarray(aug),
        "warm": np.ascontiguousarray(warm),
    }


def _fix_side(mins, qs, cs):
    """Posterior exactness check + exact host fixup for unproven rows.

    mins: device banded row minima (full P scale) for sorted queries qs
    against sorted candidates cs. Returns exact per-row minima.
    """
    i = np.arange(N) // 128
    lo = np.clip(128 * i + 64 - W // 2, 0, N - W)
    hi = lo + W
    lb = np.full(N, np.inf)
    has_l = lo > 0
    lb[has_l] = np.maximum(0.0, qs[has_l, 0] - cs[lo[has_l] - 1, 0]) ** 2
    has_r = hi < N
    lb[has_r] = np.minimum(
        lb[has_r], np.maximum(0.0, cs[np.minimum(hi[has_r], N - 1), 0] - qs[has_r, 0]) ** 2
    )
    unproven = mins > lb - 1e-5
    if unproven.any():
        # Exact windowed rescan: the true NN of row r has dist^2 <= mins[r],
        # hence coord0 within +-sqrt(mins[r]) — scan just that slice.
        rows = np.where(unproven)[0]
        c64 = cs.astype(np.float64)
        c0 = c64[:, 0]
        out = mins.copy()
        for r in rows:
            rad = float(np.sqrt(max(mins[r], 0.0) + 1e-6))
            jlo = int(np.searchsorted(c0, qs[r, 0] - rad, "left"))
            jhi = int(np.searchsorted(c0, qs[r, 0] + rad, "right"))
            if jhi > jlo:
                d = c64[jlo:jhi] - qs[r].astype(np.float64)
                out[r] = min(out[r], np.float32((d * d).sum(1).min()))
        return out
    return mins


def _postprocess(results, meta):
    """Combine per-core device outputs into the final scalar."""
    total = 0.0
    for b in range(B):
        xs, ys = meta[b]
        m = results[b]["mins"]  # [128, 2*NBLK]; [p, s*NBLK+i] = min for rank 128*i+p
        mx = 2.0 * np.ascontiguousarray(m[:, :NBLK].T).reshape(N)  # x queries vs y
        my = 2.0 * np.ascontiguousarray(m[:, NBLK:].T).reshape(N)  # y queries vs x
        mx = _fix_side(mx, xs, ys)
        my = _fix_side(my, ys, xs)
        total += mx.mean(dtype=np.float64) + my.mean(dtype=np.float64)
    return np.array(total / B, dtype=np.float32)


def _run(inputs, trace=False):
    p1 = np.ascontiguousarray(np.asarray(inputs["p1"], dtype=np.float32))
    p2 = np.ascontiguousarray(np.asarray(inputs["p2"], dtype=np.float32))
    assert p1.shape == (B, N, D) and p2.shape == (B, N, D)

    in_maps = []
    meta = []
    for b in range(B):
        xs, ys, im = _prep_batch(p1[b], p2[b])
        in_maps.append(im)
        meta.append((xs, ys))

    from concourse.bass_utils import run_bass_kernel_spmd

    nc = _build_nc()
    kw = {}
    if trace:
        kw = dict(trace=True, trace_cores=list(range(N_CORES)))
    res = run_bass_kernel_spmd(nc, in_maps, list(range(N_CORES)), **kw)
    return _postprocess(res.results, meta), res


def kernel(**inputs):
    out, _ = _run(inputs, trace=False)
    return out


def kernel_traced(**inputs):
    """Same as kernel() but also returns BassKernelResults with NTFF timing."""
    return _run(inputs, trace=True)
